# revision 48
# baseline (speedup 1.0000x reference)
"""BART decoder layer on 8 TRN2 NeuronCores.

Sharding: data-parallel over (batch, query-half): core c handles batch c//2,
query rows [half*512, half*512+512). Each core computes the full decoder layer
for its 512 query tokens; self/cross K,V are recomputed per core from the full
batch sequence (no collectives).

On-device layout is "transposed": activations live as [feature, token] so every
matmul contracts along the SBUF partition axis. Q/K/V projections run in fp8
(e4m3) with MatmulPerfMode.DoubleRow (2 reduction chunks per pass, 2x rate);
weights are host-quantized per-output-row, activations per-tensor, and dequant
scales fold into the existing bias-add ops. Cross-attention P*V also runs fp8
DoubleRow: exp() writes fp8 pT directly (output scale folded into the exp bias)
and the softmax normalization cancels both the P and V scales. Self-attention
scores/PV and out-proj/FFN stay bf16 (error budget). Accumulation is f32 in
PSUM, residuals/LayerNorm are f32. Softmax skips max-subtraction; row sums come
from an extra ones-column appended to V. LayerNorm partition-axis sums use
ones-matmuls on the TensorEngine.
"""

import sys

sys.path.insert(0, "/opt/trn_rl_repo")

import ml_dtypes
import numpy as np

import concourse.bacc as bacc
import concourse.bass as bass
import concourse.mybir as mybir
import concourse.tile as tile

BF = mybir.dt.bfloat16
F32 = mybir.dt.float32
E4 = mybir.dt.float8e4
E4np = ml_dtypes.float8_e4m3
P = 128
Act = mybir.ActivationFunctionType
Alu = mybir.AluOpType
DR = mybir.MatmulPerfMode.DoubleRow

SP = 2.0    # cross pT storage scale: pT = exp(score) * SP (scores stay < ~4.8)
SV = 16.0   # cross vsb storage scale: vsb = V * SV
SY = 16.0   # y1b (cross-attn q source) storage scale
LOG_SP = float(np.log(SP))

# causal query striping: blocks {0,3,4,7}/{1,2,5,6} per core half; columns
# [0,256) then only ever need keys [0,512) (4 k-chunks), columns [256,512)
# need all 8 -- the skip pattern is the same static program on every core
STRIPES = ([0, 3, 4, 7], [1, 2, 5, 6])

# const-column blob layout: one DMA instead of ~22 serialized descriptor issues
COLS = [("sa_bq", 8), ("sa_bk", 8), ("sa_bo", 8), ("ca_bq", 8), ("ca_bk", 8),
        ("ca_bo", 8), ("fc2_b", 8), ("ln1_g", 8), ("ln1_b", 8), ("ln1_gq", 8),
        ("ln1_bq", 8), ("ln2_g", 8), ("ln2_b", 8), ("ln3_g", 8), ("ln3_b", 8),
        ("sa_dqq", 8), ("sa_dqk", 8), ("ca_dqq", 8), ("ca_dqk", 8),
        ("sa_dqv", 1), ("ca_dqv", 1), ("fc1_b", 32)]
NB = sum(w for _, w in COLS)


def default_cfg():
    return dict(B=4, T=1024, S=1024, D=1024, H=16, F=4096, eps=1e-5,
                gelu=Act.Gelu, self_mask=True, cross_mask=False, causal=True)


def _attention(nc, pa, pools, cfg, kv_dram, L, q_sb, msk_dram, res_sb,
               prm, h_f32, pg):
    """One multi-head attention block, fully in transposed layout.

    pa: phase-scoped SBUF pool.
    kv_dram: [D, L] fp8 dram AP (source tokens for K/V)
    q_sb:    [P, DC, NQ] fp8 sbuf (source for Q)
    msk_dram:[L, NQ] bf16 dram AP of exp(mask) factors, or None (no masking)
    res_sb:  [P, DC, NQ] f32 sbuf (residual)
    prm: dict with weight dram APs (wq/wk fp8 od-tiles, wv fp8 [D,D], wo bf16
         od-tiles), bias cols, dequant cols (dqq/dqk [P,DC], dqv [P,1]) and
         fp8_pv flag. When fp8_pv: pT/vsb are fp8 and PV runs DoubleRow.
    h_f32:   [P, DC, NQ] f32 sbuf out (attn_out + bias + residual)
    Returns (h_bf_tiles, sq_tiles) lists used by LayerNorm stats.
    """
    D, H, NQ = cfg["D"], cfg["H"], cfg["NQ"]
    HD = D // H
    DC, LC = D // P, L // P
    HPC = P // HD  # heads per 128-row chunk
    KB = min(512, L)  # K-proj column block
    VB = min(512, D)  # V-proj column block
    pw, pps, psa, ppv, psm = (pools[k] for k in
                              ("w", "ps_proj", "ps_score", "ps_pv", "small"))
    fp8_pv = prm["fp8_pv"]
    causal = prm.get("causal", False)
    NQH = NQ // 2
    wq_d, wk_d, wv_d, wo_d = prm["wq"], prm["wk"], prm["wv"], prm["wo"]
    bq_c, bk_c, bo_c = prm["bq"], prm["bk"], prm["bo"]
    dqq_c, dqk_c, dqv_c = prm["dqq"], prm["dqk"], prm["dqv"]
    bg = list(prm.get("bg") or [])

    if prm.get("pre_kv") is not None:
        kv_sb = prm["pre_kv"]
    else:
        # chunked load so the first K-proj matmul starts after chunk 0 lands
        wk_first = pw.tile([P, DC, P], E4, tag="wod8", bufs=3)
        nc.sync.dma_start(wk_first[:], wk_d[0])
        kv_sb = pa.tile([P, DC, L], E4, tag="kvsrc")
        kv_r = kv_dram.rearrange("(c p) n -> p c n", p=P)
        for c in range(DC):
            eng = nc.sync if c % 2 == 0 else nc.scalar
            eng.dma_start(kv_sb[:, c, :], kv_r[:, c, :])
    if msk_dram is not None:
        msk_sb = pa.tile([P, LC, NQ], BF, tag="msk")
        nc.gpsimd.dma_start(msk_sb[:], msk_dram.rearrange("(c p) n -> p c n", p=P))

    # V bias broadcast row -> [P, D] (pre-scaled by SV when fp8_pv)
    vbias = pa.tile([P, D], BF, tag="vbias")
    nc.gpsimd.partition_broadcast(vbias[:], prm["bv_row"][:1, :])

    # K^T [d_out, k]  (ksb[p, od, k]); fp8 DoubleRow over chunk pairs
    if prm.get("pre_ksb") is not None:
        ksb = prm["pre_ksb"]
    else:
        ksb = pa.tile([P, DC, L], BF, tag="ksb")
        for od in range(DC):
            if od == 0:
                wk_t = wk_first
            else:
                wk_t = pw.tile([P, DC, P], E4, tag="wod8", bufs=3)
                nc.sync.dma_start(wk_t[:], wk_d[od])
            for nb in range(L // KB):
                ps = pps.tile([P, KB], F32, tag="proj")
                for c in range(0, DC, 2):
                    nc.tensor.matmul(ps[:], wk_t[:, c:c + 2, :],
                                     kv_sb[:, c:c + 2, nb * KB:(nb + 1) * KB],
                                     start=(c == 0), stop=(c == DC - 2),
                                     perf_mode=DR)
                nc.vector.tensor_scalar(ksb[:, od, nb * KB:(nb + 1) * KB],
                                        ps[:], dqk_c[:, od:od + 1],
                                        bk_c[:, od:od + 1], Alu.mult, Alu.add)

    # V natural layout + ones column: vsb[p, t, h*65:(h+1)*65], col 64 = SV*1
    wv_sb = pw.tile([P, DC, D], E4, tag="wvfull", bufs=1)
    nc.gpsimd.dma_start(wv_sb[:], wv_d.rearrange("(c p) n -> p c n", p=P))
    vdt = E4 if fp8_pv else BF
    vsb = pa.tile([P, LC, H * (HD + 1)], vdt, tag="vsb")
    vsb_r = vsb[:].rearrange("p t (h w) -> p t h w", w=HD + 1)
    for tt in range(LC):
        nc.vector.memset(vsb_r[:, tt, :, HD:HD + 1], SV if fp8_pv else 1.0)
        for nb in range(D // VB):
            ps = pps.tile([P, VB], F32, tag="proj")
            for c in range(0, DC, 2):
                nc.tensor.matmul(ps[:], kv_sb[:, c:c + 2, tt * P:(tt + 1) * P],
                                 wv_sb[:, c:c + 2, nb * VB:(nb + 1) * VB],
                                 start=(c == 0), stop=(c == DC - 2),
                                 perf_mode=DR)
            hpb = VB // HD  # heads per block
            nc.vector.scalar_tensor_tensor(
                vsb_r[:, tt, nb * hpb:(nb + 1) * hpb, :HD],
                ps[:].rearrange("p (h w) -> p h w", w=HD),
                dqv_c[:, 0:1],
                vbias[:, nb * VB:(nb + 1) * VB].rearrange("p (h w) -> p h w", w=HD),
                Alu.mult, Alu.add)

    # Q^T [d_out, q]; fp8 DoubleRow
    qsb = pa.tile([P, DC, NQ], BF, tag="qsb")
    for od in range(DC):
        wq_t = pw.tile([P, DC, P], E4, tag="wod8", bufs=3)
        nc.sync.dma_start(wq_t[:], wq_d[od])
        ps = pps.tile([P, NQ], F32, tag="proj")
        for c in range(0, DC, 2):
            nc.tensor.matmul(ps[:], wq_t[:, c:c + 2, :], q_sb[:, c:c + 2, :],
                             start=(c == 0), stop=(c == DC - 2), perf_mode=DR)
        nc.vector.tensor_scalar(qsb[:, od, :], ps[:], dqq_c[:, od:od + 1],
                                bq_c[:, od:od + 1], Alu.mult, Alu.add)

    # attention per head -> osb [d, q] bf16 (unnormalized; batch-normalized
    # below). With causal striping, scores/exp for kc>=LC/2 only cover query
    # columns [NQH,NQ); the untouched pT quadrant is zeroed once per buffer so
    # the full-range PV accumulation reads zeros there.
    osb = pa.tile([P, DC, NQ], BF, tag="osb")
    # compact pT: with causal, kc>=LC/2 rows only hold query columns
    # [NQH,NQ) -- the dead quadrant is never stored, and PV accumulates the
    # right half only for those chunks (first matmul zeroes the whole bank)
    if causal:
        ptw = (LC // 2) * NQ + (LC // 2) * NQH
    else:
        ptw = LC * NQ

    def pt_off(kc):
        if not causal or kc < LC // 2:
            return kc * NQ
        return (LC // 2) * NQ + (kc - LC // 2) * NQH

    pt_a = pa.tile([P, ptw], E4 if fp8_pv else BF, tag="pT0")
    pt_b = pa.tile([P, ptw], E4 if fp8_pv else BF, tag="pT1")
    pt_tiles = [pt_a, pt_b]
    for h in range(H):
        od, po = divmod(h, HPC)
        po *= HD
        pT = pt_tiles[h % 2]
        for kb in range(LC // 2):
            # two k-tiles share one 2-bank PSUM tile so exp / mask-mult run as
            # one double-width op (halves per-op fixed overhead on ACT/DVE)
            half = causal and kb >= LC // 4
            cs = slice(NQH, NQ) if half else slice(0, NQ)
            w = NQH if half else NQ
            ps_s = psa.tile([P, 2, w], F32, tag="score")
            for j in range(2):
                kc = kb * 2 + j
                nc.tensor.matmul(ps_s[:, j, :],
                                 ksb[po:po + HD, od, kc * P:(kc + 1) * P],
                                 qsb[po:po + HD, od, cs], start=True, stop=True)
            off = pt_off(kb * 2)
            ptv = pT[:, off:off + 2 * w].rearrange("p (j n) -> p j n", j=2)
            if msk_dram is not None:
                sc = pa.tile([P, 2, w], BF, tag="sc", bufs=2)
                nc.scalar.activation(sc[:], ps_s[:], Act.Exp)
                nc.vector.tensor_tensor(ptv[:, :, :], sc[:],
                                        msk_sb[:, kb * 2:(kb + 1) * 2, cs],
                                        Alu.mult)
            else:
                # fp8 pT: output scale SP folded into the exp bias
                nc.scalar.activation(ptv[:, :, :], ps_s[:], Act.Exp,
                                     bias=pools["logsp"][:, 0:1] if fp8_pv
                                     else 0.0)
            if bg:
                bg.pop(0)()
        ps_o = ppv.tile([HD + 1, NQ], F32, tag="pv")
        if fp8_pv:
            # fp8 DoubleRow over k-chunk pairs; SP*SV cancels in normalization
            for kc in range(0, LC, 2):
                ptv = pT[:, kc * NQ:(kc + 2) * NQ].rearrange(
                    "p (j n) -> p j n", j=2)
                nc.tensor.matmul(ps_o[:],
                                 vsb[:, kc:kc + 2, h * (HD + 1):(h + 1) * (HD + 1)],
                                 ptv[:, :, :],
                                 start=(kc == 0), stop=(kc == LC - 2),
                                 perf_mode=DR)
        else:
            for kc in range(LC):
                off = pt_off(kc)
                if causal and kc >= LC // 2:
                    nc.tensor.matmul(ps_o[:, NQH:],
                                     vsb[:, kc, h * (HD + 1):(h + 1) * (HD + 1)],
                                     pT[:, off:off + NQH], start=False,
                                     stop=(kc == LC - 1),
                                     skip_group_check=True)
                else:
                    nc.tensor.matmul(ps_o[:],
                                     vsb[:, kc, h * (HD + 1):(h + 1) * (HD + 1)],
                                     pT[:, off:off + NQ], start=(kc == 0),
                                     stop=(not causal and kc == LC - 1),
                                     skip_group_check=causal)
        rl = psm.tile([1, NQ], F32, tag="rl", bufs=2)
        if cfg.get("recip_fast", True):
            # custom-DVE ops misread PSUM at partition offset 64; stage the
            # denominator row to a partition-0 SBUF tile first
            rls = psm.tile([1, NQ], F32, tag="rls", bufs=2)
            # standard DVE op: safe on PSUM@64 (only custom-DVE ops misread);
            # keeps the staging copy off the exp-saturated ACT engine
            nc.vector.tensor_copy(rls[:], ps_o[HD:HD + 1, :])
            nc.vector.reciprocal_approx_fast(rl[:], rls[:])
        else:
            nc.vector.reciprocal(rl[:], ps_o[HD:HD + 1, :])
        rb = psm.tile([HD, NQ], F32, tag="rb", bufs=2)
        nc.gpsimd.partition_broadcast(rb[:], rl[:1, :])
        nc.vector.tensor_tensor(osb[po:po + HD, od, :], ps_o[:HD, :], rb[:],
                                Alu.mult)
        # background cross-K sub-units (single matmuls) fill this phase's
        # sub-microsecond tensor stall cycles
        if bg:
            bg.pop(0)()
    while bg:
        bg.pop(0)()

    if cfg.get("dbg"):
        pfx = cfg["_dbg_pfx"]
        for nm, t in (("ksb", ksb), ("vsb", vsb), ("qsb", qsb), ("osb", osb)):
            d = nc.declare_dram_parameter(f"dbg_{pfx}_{nm}", list(t.shape),
                                          t.dtype, isOutput=True)
            nc.sync.dma_start(d[:], t[:])

    # out-proj + bias + residual -> h_f32 (f32); emit bf16 + square tiles for LN
    h_bf_tiles, sq_tiles = [], []
    for od in range(DC):
        wo_t = pw.tile([P, DC, P], BF, tag="wod", bufs=3)
        nc.sync.dma_start(wo_t[:], wo_d[od])
        ps = pps.tile([P, NQ], F32, tag="proj")
        for c in range(DC):
            nc.tensor.matmul(ps[:], wo_t[:, c, :], osb[:, c, :],
                             start=(c == 0), stop=(c == DC - 1))
        nc.vector.scalar_tensor_tensor(h_f32[:, od, :], ps[:], bo_c[:, od:od + 1],
                                       res_sb[:, od, :], Alu.add, Alu.add)
        hb = pg.tile([P, NQ], BF, tag="rot_hbf", bufs=2)
        nc.scalar.copy(hb[:], h_f32[:, od, :])
        sq = pg.tile([P, NQ], BF, tag="rot_sq", bufs=2)
        nc.vector.tensor_tensor(sq[:], hb[:], hb[:], Alu.mult)
        h_bf_tiles.append(hb)
        sq_tiles.append(sq)
    return h_bf_tiles, sq_tiles


def _layernorm(nc, pg, pools, cfg, h_f32, h_bf_tiles, sq_tiles, g_c, b_c,
               y_f32, y_bf, gq_c=None, bq_c=None, out_dram=None):
    """y = LN(h) over the partition (feature) axis via ones-matmul stats.

    y_bf (if set) is the next matmul input; when gq_c/bq_c are given, y_bf is
    fp8 and they carry the extra output scale. out_dram (if set) receives
    y_f32 chunk-by-chunk so the final store overlaps the normalize loop.
    """
    D, NQ, eps = cfg["D"], cfg["NQ"], cfg["eps"]
    DC = D // P
    psm, pstat = pools["small"], pools["ps_score"]
    ones_bf = pools["ones_bf"]

    ps_sx = pstat.tile([1, NQ], F32, tag="score")
    for c in range(DC):
        nc.tensor.matmul(ps_sx[:], ones_bf[:], h_bf_tiles[c][:],
                         start=(c == 0), stop=(c == DC - 1))
    ps_sq = pstat.tile([1, NQ], F32, tag="score")
    for c in range(DC):
        nc.tensor.matmul(ps_sq[:], ones_bf[:], sq_tiles[c][:],
                         start=(c == 0), stop=(c == DC - 1))

    # ones_bf carries 1/D: ps_sx = mean, ps_sq = mean-of-squares
    m2 = psm.tile([1, NQ], F32, tag="m2")
    nc.scalar.square(m2[:], ps_sx[:])
    var = psm.tile([1, NQ], F32, tag="var")
    nc.vector.tensor_sub(var[:], ps_sq[:], m2[:])
    sd = psm.tile([1, NQ], F32, tag="sd")
    nc.scalar.activation(sd[:], var[:], Act.Sqrt, bias=pools["eps"][:, 0:1])
    rstd = psm.tile([1, NQ], F32, tag="rstd")
    nc.vector.reciprocal_approx_fast(rstd[:], sd[:])
    cc = psm.tile([1, NQ], F32, tag="cc")
    nc.vector.scalar_tensor_tensor(cc[:], ps_sx[:], -1.0, rstd[:], Alu.mult,
                                   Alu.mult)

    ab = psm.tile([P, NQ], F32, tag="ab")
    nc.gpsimd.partition_broadcast(ab[:], rstd[:1, :])
    cb = psm.tile([P, NQ], F32, tag="cb")
    nc.gpsimd.partition_broadcast(cb[:], cc[:1, :])

    for c in range(DC):
        t1 = pg.tile([P, NQ], F32, tag="rot_f32", bufs=2)
        nc.vector.tensor_tensor(t1[:], h_f32[:, c, :], ab[:], Alu.mult)
        nc.vector.tensor_tensor(t1[:], t1[:], cb[:], Alu.add)
        if y_bf is not None:
            # y_bf (the next matmul input) on the short path: ACT applies the
            # per-feature affine; the f32 copy for the residual goes to the
            # otherwise-idle gpsimd engine
            nc.scalar.activation(y_bf[:, c, :], t1[:], Act.Identity,
                                 bias=(bq_c if bq_c is not None else b_c)[:, c:c + 1],
                                 scale=(gq_c if gq_c is not None else g_c)[:, c:c + 1])
            nc.gpsimd.tensor_scalar(y_f32[:, c, :], t1[:], g_c[:, c:c + 1],
                                    b_c[:, c:c + 1], Alu.mult, Alu.add)
        elif out_dram is not None:
            nc.scalar.activation(y_f32[:, c, :], t1[:], Act.Identity,
                                 bias=b_c[:, c:c + 1], scale=g_c[:, c:c + 1])
            nc.sync.dma_start(out_dram[:, c, :], y_f32[:, c, :])
        else:
            nc.vector.tensor_scalar(y_f32[:, c, :], t1[:], g_c[:, c:c + 1],
                                    b_c[:, c:c + 1], Alu.mult, Alu.add)


def build_nc(cfg):
    B, T, S, D, H, F = (cfg[k] for k in "BTSDHF")
    NQ = cfg["NQ"] = T // 2
    DC, TC, SC, FC = D // P, T // P, S // P, F // P
    HD = D // H

    nc = bacc.Bacc("TRN2", target_bir_lowering=False,
                   debug=cfg.get("debug", False), num_devices=2 * B)
    dp = nc.declare_dram_parameter
    xT_d = dp("xT", [D, T], E4, isOutput=False)
    xqT_d = dp("xqT", [D, NQ], E4, isOutput=False)
    xres_d = dp("xres", [D, NQ], F32, isOutput=False)
    encT_d = dp("encT", [D, S], E4, isOutput=False)
    mskT_d = dp("emT", [T, NQ], BF, isOutput=False) if cfg["self_mask"] else None
    emskT_d = dp("cemT", [S, NQ], BF, isOutput=False) if cfg["cross_mask"] else None
    w_d = {}
    for nm in ("sa_wq", "sa_wk", "ca_wq", "ca_wk"):
        w_d[nm] = dp(nm + "T", [DC, P, DC, P], E4, isOutput=False)
    for nm in ("sa_wo", "ca_wo"):
        w_d[nm] = dp(nm + "T", [DC, P, DC, P], BF, isOutput=False)
    for nm in ("sa_wv", "ca_wv"):
        w_d[nm] = dp(nm + "T", [D, D], E4, isOutput=False)
    f1_d = dp("f1T", [FC, P, DC, P], BF, isOutput=False)
    f2_d = dp("f2T", [DC, P, FC, P], BF, isOutput=False)
    blob_d = dp("cols_blob", [P, NB], F32, isOutput=False)
    bvrow_d = dp("sa_bv_row", [1, D], BF, isOutput=False)
    cvrow_d = dp("ca_bv_row", [1, D], BF, isOutput=False)
    outT_d = dp("outT", [D, NQ], F32, isOutput=True)

    with tile.TileContext(nc) as tc:
        with tc.tile_pool(name="const", bufs=1) as pc, \
             tc.tile_pool(name="glob", bufs=1) as pg, \
             tc.tile_pool(name="wpool", bufs=1) as pw, \
             tc.tile_pool(name="small", bufs=1) as psm, \
             tc.tile_pool(name="ps_proj", bufs=2, space="PSUM") as pps, \
             tc.tile_pool(name="ps_score", bufs=2, space="PSUM") as psa, \
             tc.tile_pool(name="ps_pv", bufs=2, space="PSUM") as ppv:

            # constants
            ones_bf = pc.tile([P, 1], BF)
            nc.vector.memset(ones_bf[:], 1.0 / cfg["D"])
            ones_f32 = pc.tile([1, P], F32)
            nc.vector.memset(ones_f32[:], 1.0)
            eps_t = pc.tile([1, 1], F32, tag="eps")
            nc.vector.memset(eps_t[:], float(cfg["eps"]))
            logsp = pc.tile([P, 1], F32, tag="logsp")
            nc.vector.memset(logsp[:], LOG_SP)
            blob_t = pc.tile([P, NB], F32, tag="blob")
            nc.sync.dma_start(blob_t[:], blob_d[:])
            bc_sb = {}
            off = 0
            for nm, w in COLS:
                bc_sb[nm] = blob_t[:, off:off + w]
                off += w
            bvrow_sb = pc.tile([1, D], BF, tag="bvrow_sa")
            nc.gpsimd.dma_start(bvrow_sb[:], bvrow_d[:])
            cvrow_sb = pc.tile([1, D], BF, tag="bvrow_ca")
            nc.gpsimd.dma_start(cvrow_sb[:], cvrow_d[:])

            pools = dict(w=pw, small=psm, ps_proj=pps, ps_score=psa,
                         ps_pv=ppv, ones_bf=ones_bf, ones_f32=ones_f32,
                         logsp=logsp, eps=eps_t)

            # globals: residual-chain f32 slots and q-source fp8 slots
            xq_sb = pg.tile([P, DC, NQ], E4, tag="qsrc8", bufs=1)
            nc.gpsimd.dma_start(xq_sb[:], xqT_d.rearrange("(c p) n -> p c n", p=P))
            xres_sb = pg.tile([P, DC, NQ], F32, tag="af32", bufs=2)
            nc.gpsimd.dma_start(xres_sb[:], xres_d.rearrange("(c p) n -> p c n", p=P))

            # cross-attention K-proj hoisted as background units: input-only
            # deps, run interleaved into the self-attention core phase where
            # the tensor engine otherwise stalls on pT/softmax dependencies
            enc_sb = pg.tile([P, DC, S], E4, tag="encsb", bufs=1)
            enc_r = encT_d.rearrange("(c p) n -> p c n", p=P)
            for c in range(DC):
                nc.gpsimd.dma_start(enc_sb[:, c, :], enc_r[:, c, :])
            cksb = pg.tile([P, DC, S], BF, tag="cksb", bufs=1)
            KBX = min(512, S)
            ck_hold = {}

            def _ck_step(od, nb, c):
                def run():
                    if nb == 0 and c == 0:
                        wk_t = pw.tile([P, DC, P], E4, tag="wod8", bufs=3)
                        nc.sync.dma_start(wk_t[:], w_d["ca_wk"][od])
                        ck_hold["wk"] = wk_t
                    if c == 0:
                        ps = pps.tile([P, KBX], F32, tag="proj")
                        ck_hold[(od, nb)] = ps
                    ps = ck_hold[(od, nb)]
                    nc.tensor.matmul(ps[:], ck_hold["wk"][:, c:c + 2, :],
                                     enc_sb[:, c:c + 2,
                                            nb * KBX:(nb + 1) * KBX],
                                     start=(c == 0), stop=(c == DC - 2),
                                     perf_mode=DR)
                return run

            def _ck_dq(od, nb):
                def run():
                    ps = ck_hold.pop((od, nb))
                    nc.vector.tensor_scalar(cksb[:, od, nb * KBX:(nb + 1) * KBX],
                                            ps[:], bc_sb["ca_dqk"][:, od:od + 1],
                                            bc_sb["ca_bk"][:, od:od + 1],
                                            Alu.mult, Alu.add)
                return run

            # 80 sub-unit closures consumed one per score-block/normalize slot
            ck_q = []
            for od in range(DC):
                for nb in range(S // KBX):
                    for c in range(0, DC, 2):
                        ck_q.append(_ck_step(od, nb, c))
                    ck_q.append(_ck_dq(od, nb))

            causal = cfg["self_mask"] and cfg.get("causal", False)
            sa_prm = dict(wq=w_d["sa_wq"], wk=w_d["sa_wk"], wv=w_d["sa_wv"],
                          wo=w_d["sa_wo"], bq=bc_sb["sa_bq"], bk=bc_sb["sa_bk"],
                          bo=bc_sb["sa_bo"], dqq=bc_sb["sa_dqq"],
                          dqk=bc_sb["sa_dqk"], dqv=bc_sb["sa_dqv"],
                          bv_row=bvrow_sb, fp8_pv=not cfg["self_mask"],
                          causal=causal, bg=ck_q)
            ca_prm = dict(wq=w_d["ca_wq"], wk=w_d["ca_wk"], wv=w_d["ca_wv"],
                          wo=w_d["ca_wo"], bq=bc_sb["ca_bq"], bk=bc_sb["ca_bk"],
                          bo=bc_sb["ca_bo"], dqq=bc_sb["ca_dqq"],
                          dqk=bc_sb["ca_dqk"], dqv=bc_sb["ca_dqv"],
                          bv_row=cvrow_sb, fp8_pv=not cfg["cross_mask"],
                          pre_kv=enc_sb, pre_ksb=cksb)

            # ---- self attention + LN1 ----
            h1 = pg.tile([P, DC, NQ], F32, tag="af32", bufs=2)
            cfg["_dbg_pfx"] = "sa"
            with tc.tile_pool(name="attn1", bufs=1) as pa:
                hbf, sq = _attention(nc, pa, pools, cfg, xT_d, T, xq_sb, mskT_d,
                                     xres_sb, sa_prm, h1, pg)
                y1 = pg.tile([P, DC, NQ], F32, tag="af32", bufs=2)
                y1b = pg.tile([P, DC, NQ], E4, tag="qsrc8", bufs=1)
                _layernorm(nc, pg, pools, cfg, h1, hbf, sq,
                           bc_sb["ln1_g"], bc_sb["ln1_b"], y1, y1b,
                           gq_c=bc_sb["ln1_gq"], bq_c=bc_sb["ln1_bq"])

            # ---- cross attention + LN2 ----
            h2 = pg.tile([P, DC, NQ], F32, tag="af32", bufs=2)
            cfg["_dbg_pfx"] = "ca"
            with tc.tile_pool(name="attn2", bufs=1) as pa:
                hbf, sq = _attention(nc, pa, pools, cfg, encT_d, S, y1b, emskT_d,
                                     y1, ca_prm, h2, pg)
                y2 = pg.tile([P, DC, NQ], F32, tag="af32", bufs=2)
                y2b = pg.tile([P, DC, NQ], BF, tag="qsrc", bufs=1)
                _layernorm(nc, pg, pools, cfg, h2, hbf, sq,
                           bc_sb["ln2_g"], bc_sb["ln2_b"], y2, y2b)

            if cfg.get("dbg"):
                for nm, t in (("h1", h1), ("y1", y1), ("h2", h2), ("y2", y2)):
                    d = dp(f"dbg_{nm}", list(t.shape), t.dtype, isOutput=True)
                    nc.sync.dma_start(d[:], t[:])

            # ---- FFN + LN3 ----
            with tc.tile_pool(name="ffn", bufs=1) as pa:
                fsb = pa.tile([P, FC, NQ], BF, tag="fsb")
                for ft in range(FC):
                    w1 = pw.tile([P, DC, P], BF, tag="wod", bufs=3)
                    nc.sync.dma_start(w1[:], f1_d[ft])
                    ps = pps.tile([P, NQ], F32, tag="proj")
                    for c in range(DC):
                        nc.tensor.matmul(ps[:], w1[:, c, :], y2b[:, c, :],
                                         start=(c == 0), stop=(c == DC - 1))
                    nc.scalar.activation(fsb[:, ft, :], ps[:], cfg["gelu"],
                                         bias=bc_sb["fc1_b"][:, ft:ft + 1])
                h3 = pg.tile([P, DC, NQ], F32, tag="af32", bufs=2)
                hbf, sq = [], []
                for od in range(DC):
                    w2 = pa.tile([P, FC, P], BF, tag="w2", bufs=2)
                    nc.sync.dma_start(w2[:], f2_d[od])
                    ps = pps.tile([P, NQ], F32, tag="proj")
                    for fc_ in range(FC):
                        nc.tensor.matmul(ps[:], w2[:, fc_, :], fsb[:, fc_, :],
                                         start=(fc_ == 0), stop=(fc_ == FC - 1))
                    nc.vector.scalar_tensor_tensor(h3[:, od, :], ps[:],
                                                   bc_sb["fc2_b"][:, od:od + 1],
                                                   y2[:, od, :], Alu.add, Alu.add)
                    hb = pg.tile([P, NQ], BF, tag="rot_hbf", bufs=2)
                    nc.scalar.copy(hb[:], h3[:, od, :])
                    s2 = pg.tile([P, NQ], BF, tag="rot_sq", bufs=2)
                    nc.vector.tensor_tensor(s2[:], hb[:], hb[:], Alu.mult)
                    hbf.append(hb)
                    sq.append(s2)
                out_f = pg.tile([P, DC, NQ], F32, tag="af32", bufs=2)
                _layernorm(nc, pg, pools, cfg, h3, hbf, sq,
                           bc_sb["ln3_g"], bc_sb["ln3_b"], out_f, None,
                           out_dram=outT_d.rearrange("(c p) n -> p c n", p=P))

    nc.compile()
    return nc


def make_in_maps(cfg, inputs):
    B, T, S, D, H, F = (cfg[k] for k in "BTSDHF")
    NQ = T // 2
    DC, FC = D // P, F // P
    HD = D // H
    bf = ml_dtypes.bfloat16

    def col(v):  # [D'] -> [P, D'//P]
        return np.ascontiguousarray(np.asarray(v, np.float32).reshape(-1, P).T)

    def wtile(w):  # [DO, DI] -> [DO/P, P, DI/P, P] od-tiles of transposed weight
        w = np.asarray(w, np.float32)
        do, di = w.shape
        return np.ascontiguousarray(
            w.reshape(do // P, P, di // P, P).transpose(0, 3, 2, 1)).astype(bf)

    def wtile8(w):  # fp8 od-tiles + per-output-row dequant (absmax/240)
        w = np.asarray(w, np.float32)
        do, di = w.shape
        am = np.abs(w).max(axis=1)
        s = 240.0 / np.maximum(am, 1e-30)
        w8 = (w * s[:, None]).astype(E4np)
        t = np.ascontiguousarray(
            w8.reshape(do // P, P, di // P, P).transpose(0, 3, 2, 1))
        return t, (1.0 / s).astype(np.float32)

    x_np = np.asarray(inputs["hidden_states"], np.float32)
    enc_np = np.asarray(inputs["encoder_hidden_states"], np.float32)
    s_x = 240.0 / max(float(np.abs(x_np).max()), 1e-30)
    s_enc = 240.0 / max(float(np.abs(enc_np).max()), 1e-30)

    shared = {}
    sc = HD ** -0.5
    t, dq = wtile8(np.asarray(inputs["sa_wq"]) * sc)
    cols = {}
    shared["sa_wqT"] = t
    cols["sa_dqq"] = col(dq / s_x)
    t, dq = wtile8(np.asarray(inputs["ca_wq"]) * sc)
    shared["ca_wqT"] = t
    cols["ca_dqq"] = col(dq / SY)
    t, dq = wtile8(inputs["sa_wk"])
    shared["sa_wkT"] = t
    cols["sa_dqk"] = col(dq / s_x)
    t, dq = wtile8(inputs["ca_wk"])
    shared["ca_wkT"] = t
    cols["ca_dqk"] = col(dq / s_enc)
    for nm in ("sa_wo", "ca_wo"):
        shared[nm + "T"] = wtile(inputs[nm])
    # V weights: per-tensor scale (dequant rides the free axis -> one scalar)
    wv = np.asarray(inputs["sa_wv"], np.float32)
    s_wv = 240.0 / max(float(np.abs(wv).max()), 1e-30)
    shared["sa_wvT"] = np.ascontiguousarray((wv.T * s_wv)).astype(E4np)
    cols["sa_dqv"] = np.full((P, 1), 1.0 / (s_wv * s_x), np.float32)
    wv = np.asarray(inputs["ca_wv"], np.float32)
    s_wv = 240.0 / max(float(np.abs(wv).max()), 1e-30)
    shared["ca_wvT"] = np.ascontiguousarray((wv.T * s_wv)).astype(E4np)
    cols["ca_dqv"] = np.full((P, 1), SV / (s_wv * s_enc), np.float32)
    shared["f1T"] = wtile(inputs["fc1_w"])
    shared["f2T"] = wtile(inputs["fc2_w"])
    cols["sa_bq"] = col(np.asarray(inputs["sa_bq"], np.float32) * sc)
    cols["ca_bq"] = col(np.asarray(inputs["ca_bq"], np.float32) * sc)
    for nm in ("sa_bk", "sa_bo", "ca_bk", "ca_bo", "fc2_b", "fc1_b",
               "ln1_g", "ln1_b", "ln2_g", "ln2_b", "ln3_g", "ln3_b"):
        cols[nm] = col(inputs[nm])
    cols["ln1_gq"] = col(np.asarray(inputs["ln1_g"], np.float32) * SY)
    cols["ln1_bq"] = col(np.asarray(inputs["ln1_b"], np.float32) * SY)
    shared["cols_blob"] = np.ascontiguousarray(
        np.concatenate([cols[nm] for nm, _ in COLS], axis=1))
    shared["sa_bv_row"] = np.asarray(inputs["sa_bv"], np.float32)[None, :].astype(bf)
    shared["ca_bv_row"] = (np.asarray(inputs["ca_bv"], np.float32)[None, :]
                           * SV).astype(bf)

    causal = cfg["self_mask"] and cfg.get("causal", False)
    in_maps = []
    for c in range(2 * B):
        b, half = divmod(c, 2)
        if causal:
            qs = np.concatenate([np.arange(blk * P, (blk + 1) * P)
                                 for blk in STRIPES[half]])
        else:
            qs = np.arange(half * NQ, (half + 1) * NQ)
        x = x_np[b]  # [T, D]
        m = {}
        m.update(shared)
        m["xT"] = np.ascontiguousarray((x.T * s_x)).astype(E4np)
        m["xqT"] = np.ascontiguousarray((x[qs].T * s_x)).astype(E4np)
        m["xres"] = np.ascontiguousarray(x[qs].T)
        m["encT"] = np.ascontiguousarray((enc_np[b].T * s_enc)).astype(E4np)
        if cfg.get("self_mask", True):
            m["emT"] = np.ascontiguousarray(np.exp(
                np.asarray(inputs["attention_mask"][b, 0], np.float32)[qs].T)).astype(bf)
        if cfg.get("cross_mask", False):
            m["cemT"] = np.ascontiguousarray(np.exp(
                np.asarray(inputs["encoder_attention_mask"][b, 0], np.float32)[qs].T)).astype(bf)
        in_maps.append(m)
    return in_maps


_NC_CACHE = {}


def get_nc(cfg=None):
    cfg = cfg or default_cfg()
    key = tuple(sorted((k, str(v)) for k, v in cfg.items()))
    if key not in _NC_CACHE:
        _NC_CACHE[key] = build_nc(dict(cfg))
    return _NC_CACHE[key]


def _is_causal_mask(mask, T):
    m = np.asarray(mask)
    tri = np.arange(T)[:, None] >= np.arange(T)[None, :]
    return bool(np.all((m[:, 0] > -1.0) == tri))


def kernel(**inputs):
    from concourse.bass_utils import run_bass_kernel_spmd

    cfg = default_cfg()
    cfg["self_mask"] = bool(np.any(np.asarray(inputs["attention_mask"])))
    cfg["cross_mask"] = bool(np.any(np.asarray(inputs["encoder_attention_mask"])))
    B, T, D = cfg["B"], cfg["T"], cfg["D"]
    NQ = T // 2
    cfg["causal"] = (cfg["self_mask"]
                     and _is_causal_mask(inputs["attention_mask"], T))
    causal = cfg["self_mask"] and cfg["causal"]
    nc = get_nc(cfg)
    in_maps = make_in_maps(cfg, inputs)
    res = run_bass_kernel_spmd(nc, in_maps, list(range(2 * B))).results
    out = np.empty((B, T, D), np.float32)
    for c in range(2 * B):
        b, half = divmod(c, 2)
        if causal:
            qi = np.concatenate([np.arange(blk * P, (blk + 1) * P)
                                 for blk in STRIPES[half]])
            out[b, qi, :] = res[c]["outT"].T
        else:
            out[b, half * NQ:(half + 1) * NQ, :] = res[c]["outT"].T
    return out


# revision 49
# speedup vs baseline: 1.0117x; 1.0117x over previous
"""BART decoder layer on 8 TRN2 NeuronCores.

Sharding: data-parallel over (batch, query-half): core c handles batch c//2,
query rows [half*512, half*512+512). Each core computes the full decoder layer
for its 512 query tokens; self/cross K,V are recomputed per core from the full
batch sequence (no collectives).

On-device layout is "transposed": activations live as [feature, token] so every
matmul contracts along the SBUF partition axis. Q/K/V projections run in fp8
(e4m3) with MatmulPerfMode.DoubleRow (2 reduction chunks per pass, 2x rate);
weights are host-quantized per-output-row, activations per-tensor, and dequant
scales fold into the existing bias-add ops. Cross-attention P*V also runs fp8
DoubleRow: exp() writes fp8 pT directly (output scale folded into the exp bias)
and the softmax normalization cancels both the P and V scales. Self-attention
scores/PV and out-proj/FFN stay bf16 (error budget). Accumulation is f32 in
PSUM, residuals/LayerNorm are f32. Softmax skips max-subtraction; row sums come
from an extra ones-column appended to V. LayerNorm partition-axis sums use
ones-matmuls on the TensorEngine.
"""

import sys

sys.path.insert(0, "/opt/trn_rl_repo")

import ml_dtypes
import numpy as np

import concourse.bacc as bacc
import concourse.bass as bass
import concourse.mybir as mybir
import concourse.tile as tile

BF = mybir.dt.bfloat16
F32 = mybir.dt.float32
E4 = mybir.dt.float8e4
E4np = ml_dtypes.float8_e4m3
P = 128
Act = mybir.ActivationFunctionType
Alu = mybir.AluOpType
DR = mybir.MatmulPerfMode.DoubleRow

SP = 2.0    # cross pT storage scale: pT = exp(score) * SP (scores stay < ~4.8)
SV = 16.0   # cross vsb storage scale: vsb = V * SV
SY = 16.0   # y1b (cross-attn q source) storage scale
LOG_SP = float(np.log(SP))

# causal query striping: blocks {0,3,4,7}/{1,2,5,6} per core half; columns
# [0,256) then only ever need keys [0,512) (4 k-chunks), columns [256,512)
# need all 8 -- the skip pattern is the same static program on every core
STRIPES = ([0, 3, 4, 7], [1, 2, 5, 6])

# const-column blob layout: one DMA instead of ~22 serialized descriptor issues
COLS = [("sa_bq", 8), ("sa_bk", 8), ("sa_bo", 8), ("ca_bq", 8), ("ca_bk", 8),
        ("ca_bo", 8), ("fc2_b", 8), ("ln1_g", 8), ("ln1_b", 8), ("ln1_gq", 8),
        ("ln1_bq", 8), ("ln2_g", 8), ("ln2_b", 8), ("ln3_g", 8), ("ln3_b", 8),
        ("sa_dqq", 8), ("sa_dqk", 8), ("ca_dqq", 8), ("ca_dqk", 8),
        ("sa_dqv", 1), ("ca_dqv", 1), ("fc1_b", 32)]
NB = sum(w for _, w in COLS)


def default_cfg():
    return dict(B=4, T=1024, S=1024, D=1024, H=16, F=4096, eps=1e-5,
                gelu=Act.Gelu, self_mask=True, cross_mask=False, causal=True)


def _attention(nc, pa, pools, cfg, kv_dram, L, q_sb, msk_dram, res_sb,
               prm, h_f32, pg):
    """One multi-head attention block, fully in transposed layout.

    pa: phase-scoped SBUF pool.
    kv_dram: [D, L] fp8 dram AP (source tokens for K/V)
    q_sb:    [P, DC, NQ] fp8 sbuf (source for Q)
    msk_dram:[L, NQ] bf16 dram AP of exp(mask) factors, or None (no masking)
    res_sb:  [P, DC, NQ] f32 sbuf (residual)
    prm: dict with weight dram APs (wq/wk fp8 od-tiles, wv fp8 [D,D], wo bf16
         od-tiles), bias cols, dequant cols (dqq/dqk [P,DC], dqv [P,1]) and
         fp8_pv flag. When fp8_pv: pT/vsb are fp8 and PV runs DoubleRow.
    h_f32:   [P, DC, NQ] f32 sbuf out (attn_out + bias + residual)
    Returns (h_bf_tiles, sq_tiles) lists used by LayerNorm stats.
    """
    D, H, NQ = cfg["D"], cfg["H"], cfg["NQ"]
    HD = D // H
    DC, LC = D // P, L // P
    HPC = P // HD  # heads per 128-row chunk
    KB = min(512, L)  # K-proj column block
    VB = min(512, D)  # V-proj column block
    pw, pps, psa, ppv, psm = (pools[k] for k in
                              ("w", "ps_proj", "ps_score", "ps_pv", "small"))
    fp8_pv = prm["fp8_pv"]
    causal = prm.get("causal", False)
    NQH = NQ // 2
    wq_d, wk_d, wv_d, wo_d = prm["wq"], prm["wk"], prm["wv"], prm["wo"]
    bq_c, bk_c, bo_c = prm["bq"], prm["bk"], prm["bo"]
    dqq_c, dqk_c, dqv_c = prm["dqq"], prm["dqk"], prm["dqv"]
    bg = list(prm.get("bg") or [])

    if prm.get("pre_kv") is not None:
        kv_sb = prm["pre_kv"]
    else:
        # chunked load so the first K-proj matmul starts after chunk 0 lands
        wk_first = pw.tile([P, DC, P], E4, tag="wod8", bufs=3)
        nc.sync.dma_start(wk_first[:], wk_d[0])
        kv_sb = pa.tile([P, DC, L], E4, tag="kvsrc")
        kv_r = kv_dram.rearrange("(c p) n -> p c n", p=P)
        for c in range(DC):
            eng = nc.sync if c % 2 == 0 else nc.scalar
            eng.dma_start(kv_sb[:, c, :], kv_r[:, c, :])
    if msk_dram is not None:
        msk_sb = pa.tile([P, LC, NQ], BF, tag="msk")
        nc.gpsimd.dma_start(msk_sb[:], msk_dram.rearrange("(c p) n -> p c n", p=P))

    # V bias broadcast row -> [P, D] (pre-scaled by SV when fp8_pv)
    vbias = pa.tile([P, D], BF, tag="vbias")
    nc.gpsimd.partition_broadcast(vbias[:], prm["bv_row"][:1, :])

    # K^T [d_out, k]  (ksb[p, od, k]); fp8 DoubleRow over chunk pairs
    if prm.get("pre_ksb") is not None:
        ksb = prm["pre_ksb"]
    else:
        ksb = pa.tile([P, DC, L], BF, tag="ksb")
        for od in range(DC):
            if od == 0:
                wk_t = wk_first
            else:
                wk_t = pw.tile([P, DC, P], E4, tag="wod8", bufs=3)
                nc.sync.dma_start(wk_t[:], wk_d[od])
            for nb in range(L // KB):
                ps = pps.tile([P, KB], F32, tag="proj")
                for c in range(0, DC, 2):
                    nc.tensor.matmul(ps[:], wk_t[:, c:c + 2, :],
                                     kv_sb[:, c:c + 2, nb * KB:(nb + 1) * KB],
                                     start=(c == 0), stop=(c == DC - 2),
                                     perf_mode=DR)
                nc.vector.tensor_scalar(ksb[:, od, nb * KB:(nb + 1) * KB],
                                        ps[:], dqk_c[:, od:od + 1],
                                        bk_c[:, od:od + 1], Alu.mult, Alu.add)

    # V natural layout + ones column: vsb[p, t, h*65:(h+1)*65], col 64 = SV*1
    wv_sb = pw.tile([P, DC, D], E4, tag="wvfull", bufs=1)
    nc.gpsimd.dma_start(wv_sb[:], wv_d.rearrange("(c p) n -> p c n", p=P))
    vdt = E4 if fp8_pv else BF
    vsb = pa.tile([P, LC, H * (HD + 1)], vdt, tag="vsb")
    vsb_r = vsb[:].rearrange("p t (h w) -> p t h w", w=HD + 1)
    for tt in range(LC):
        nc.vector.memset(vsb_r[:, tt, :, HD:HD + 1], SV if fp8_pv else 1.0)
        for nb in range(D // VB):
            ps = pps.tile([P, VB], F32, tag="proj")
            for c in range(0, DC, 2):
                nc.tensor.matmul(ps[:], kv_sb[:, c:c + 2, tt * P:(tt + 1) * P],
                                 wv_sb[:, c:c + 2, nb * VB:(nb + 1) * VB],
                                 start=(c == 0), stop=(c == DC - 2),
                                 perf_mode=DR)
            hpb = VB // HD  # heads per block
            nc.vector.scalar_tensor_tensor(
                vsb_r[:, tt, nb * hpb:(nb + 1) * hpb, :HD],
                ps[:].rearrange("p (h w) -> p h w", w=HD),
                dqv_c[:, 0:1],
                vbias[:, nb * VB:(nb + 1) * VB].rearrange("p (h w) -> p h w", w=HD),
                Alu.mult, Alu.add)

    # Q^T [d_out, q]; fp8 DoubleRow
    qsb = pa.tile([P, DC, NQ], BF, tag="qsb")
    for od in range(DC):
        wq_t = pw.tile([P, DC, P], E4, tag="wod8", bufs=3)
        nc.sync.dma_start(wq_t[:], wq_d[od])
        ps = pps.tile([P, NQ], F32, tag="proj")
        for c in range(0, DC, 2):
            nc.tensor.matmul(ps[:], wq_t[:, c:c + 2, :], q_sb[:, c:c + 2, :],
                             start=(c == 0), stop=(c == DC - 2), perf_mode=DR)
        nc.vector.tensor_scalar(qsb[:, od, :], ps[:], dqq_c[:, od:od + 1],
                                bq_c[:, od:od + 1], Alu.mult, Alu.add)

    # attention per head -> osb [d, q] bf16 (unnormalized; batch-normalized
    # below). With causal striping, scores/exp for kc>=LC/2 only cover query
    # columns [NQH,NQ); the untouched pT quadrant is zeroed once per buffer so
    # the full-range PV accumulation reads zeros there.
    osb = pa.tile([P, DC, NQ], BF, tag="osb")
    # compact pT: with causal, kc>=LC/2 rows only hold query columns
    # [NQH,NQ) -- the dead quadrant is never stored, and PV accumulates the
    # right half only for those chunks (first matmul zeroes the whole bank)
    if causal:
        ptw = (LC // 2) * NQ + (LC // 2) * NQH
    else:
        ptw = LC * NQ

    def pt_off(kc):
        if not causal or kc < LC // 2:
            return kc * NQ
        return (LC // 2) * NQ + (kc - LC // 2) * NQH

    pt_a = pa.tile([P, ptw], E4 if fp8_pv else BF, tag="pT0")
    pt_b = pa.tile([P, ptw], E4 if fp8_pv else BF, tag="pT1")
    pt_tiles = [pt_a, pt_b]
    for h in range(H):
        od, po = divmod(h, HPC)
        po *= HD
        pT = pt_tiles[h % 2]
        for kb in range(LC // 2):
            # two k-tiles share one 2-bank PSUM tile so exp / mask-mult run as
            # one double-width op (halves per-op fixed overhead on ACT/DVE)
            half = causal and kb >= LC // 4
            cs = slice(NQH, NQ) if half else slice(0, NQ)
            w = NQH if half else NQ
            ps_s = psa.tile([P, 2, w], F32, tag="score")
            for j in range(2):
                kc = kb * 2 + j
                nc.tensor.matmul(ps_s[:, j, :],
                                 ksb[po:po + HD, od, kc * P:(kc + 1) * P],
                                 qsb[po:po + HD, od, cs], start=True, stop=True)
            off = pt_off(kb * 2)
            ptv = pT[:, off:off + 2 * w].rearrange("p (j n) -> p j n", j=2)
            if msk_dram is not None:
                sc = pa.tile([P, 2, w], BF, tag="sc", bufs=2)
                nc.scalar.activation(sc[:], ps_s[:], Act.Exp)
                nc.vector.tensor_tensor(ptv[:, :, :], sc[:],
                                        msk_sb[:, kb * 2:(kb + 1) * 2, cs],
                                        Alu.mult)
            else:
                # fp8 pT: output scale SP folded into the exp bias
                nc.scalar.activation(ptv[:, :, :], ps_s[:], Act.Exp,
                                     bias=pools["logsp"][:, 0:1] if fp8_pv
                                     else 0.0)
            if bg:
                bg.pop(0)()
        ps_o = ppv.tile([HD + 1, NQ], F32, tag="pv")
        if fp8_pv:
            # fp8 DoubleRow over k-chunk pairs; SP*SV cancels in normalization
            for kc in range(0, LC, 2):
                ptv = pT[:, kc * NQ:(kc + 2) * NQ].rearrange(
                    "p (j n) -> p j n", j=2)
                nc.tensor.matmul(ps_o[:],
                                 vsb[:, kc:kc + 2, h * (HD + 1):(h + 1) * (HD + 1)],
                                 ptv[:, :, :],
                                 start=(kc == 0), stop=(kc == LC - 2),
                                 perf_mode=DR)
        else:
            for kc in range(LC):
                off = pt_off(kc)
                if causal and kc >= LC // 2:
                    nc.tensor.matmul(ps_o[:, NQH:],
                                     vsb[:, kc, h * (HD + 1):(h + 1) * (HD + 1)],
                                     pT[:, off:off + NQH], start=False,
                                     stop=(kc == LC - 1),
                                     skip_group_check=True)
                else:
                    nc.tensor.matmul(ps_o[:],
                                     vsb[:, kc, h * (HD + 1):(h + 1) * (HD + 1)],
                                     pT[:, off:off + NQ], start=(kc == 0),
                                     stop=(not causal and kc == LC - 1),
                                     skip_group_check=causal)
        rl = psm.tile([1, NQ], F32, tag="rl", bufs=2)
        if cfg.get("recip_fast", True):
            # custom-DVE ops misread PSUM at partition offset 64; stage the
            # denominator row to a partition-0 SBUF tile first
            rls = psm.tile([1, NQ], F32, tag="rls", bufs=2)
            # standard DVE op: safe on PSUM@64 (only custom-DVE ops misread);
            # keeps the staging copy off the exp-saturated ACT engine
            nc.vector.tensor_copy(rls[:], ps_o[HD:HD + 1, :])
            nc.vector.reciprocal_approx_fast(rl[:], rls[:])
        else:
            nc.vector.reciprocal(rl[:], ps_o[HD:HD + 1, :])
        rb = psm.tile([HD, NQ], F32, tag="rb", bufs=2)
        nc.gpsimd.partition_broadcast(rb[:], rl[:1, :])
        nc.vector.tensor_tensor(osb[po:po + HD, od, :], ps_o[:HD, :], rb[:],
                                Alu.mult)
        # background cross-K sub-units (single matmuls) fill this phase's
        # sub-microsecond tensor stall cycles
        if bg:
            bg.pop(0)()
    while bg:
        bg.pop(0)()

    if cfg.get("dbg"):
        pfx = cfg["_dbg_pfx"]
        for nm, t in (("ksb", ksb), ("vsb", vsb), ("qsb", qsb), ("osb", osb)):
            d = nc.declare_dram_parameter(f"dbg_{pfx}_{nm}", list(t.shape),
                                          t.dtype, isOutput=True)
            nc.sync.dma_start(d[:], t[:])

    # out-proj + bias + residual -> h_f32 (f32); emit bf16 + square tiles for LN
    h_bf_tiles, sq_tiles = [], []
    for od in range(DC):
        wo_t = pw.tile([P, DC, P], BF, tag="wod", bufs=3)
        nc.sync.dma_start(wo_t[:], wo_d[od])
        ps = pps.tile([P, NQ], F32, tag="proj")
        for c in range(DC):
            nc.tensor.matmul(ps[:], wo_t[:, c, :], osb[:, c, :],
                             start=(c == 0), stop=(c == DC - 1))
        nc.vector.scalar_tensor_tensor(h_f32[:, od, :], ps[:], bo_c[:, od:od + 1],
                                       res_sb[:, od, :], Alu.add, Alu.add)
        hb = pg.tile([P, NQ], BF, tag="rot_hbf", bufs=2)
        nc.scalar.copy(hb[:], h_f32[:, od, :])
        sq = pg.tile([P, NQ], BF, tag="rot_sq", bufs=2)
        nc.vector.tensor_tensor(sq[:], hb[:], hb[:], Alu.mult)
        h_bf_tiles.append(hb)
        sq_tiles.append(sq)
    return h_bf_tiles, sq_tiles


def _layernorm(nc, pg, pools, cfg, h_f32, h_bf_tiles, sq_tiles, g_c, b_c,
               y_f32, y_bf, gq_c=None, bq_c=None, out_dram=None):
    """y = LN(h) over the partition (feature) axis via ones-matmul stats.

    y_bf (if set) is the next matmul input; when gq_c/bq_c are given, y_bf is
    fp8 and they carry the extra output scale. out_dram (if set) receives
    y_f32 chunk-by-chunk so the final store overlaps the normalize loop.
    """
    D, NQ, eps = cfg["D"], cfg["NQ"], cfg["eps"]
    DC = D // P
    psm, pstat = pools["small"], pools["ps_score"]
    ones_bf = pools["ones_bf"]

    ps_sx = pstat.tile([1, NQ], F32, tag="score")
    for c in range(DC):
        nc.tensor.matmul(ps_sx[:], ones_bf[:], h_bf_tiles[c][:],
                         start=(c == 0), stop=(c == DC - 1))
    ps_sq = pstat.tile([1, NQ], F32, tag="score")
    for c in range(DC):
        nc.tensor.matmul(ps_sq[:], ones_bf[:], sq_tiles[c][:],
                         start=(c == 0), stop=(c == DC - 1))

    # ones_bf carries 1/D: ps_sx = mean, ps_sq = mean-of-squares
    m2 = psm.tile([1, NQ], F32, tag="m2")
    nc.scalar.square(m2[:], ps_sx[:])
    var = psm.tile([1, NQ], F32, tag="var")
    nc.vector.tensor_sub(var[:], ps_sq[:], m2[:])
    sd = psm.tile([1, NQ], F32, tag="sd")
    nc.scalar.activation(sd[:], var[:], Act.Sqrt, bias=pools["eps"][:, 0:1])
    rstd = psm.tile([1, NQ], F32, tag="rstd")
    nc.vector.reciprocal_approx_fast(rstd[:], sd[:])
    cc = psm.tile([1, NQ], F32, tag="cc")
    nc.vector.scalar_tensor_tensor(cc[:], ps_sx[:], -1.0, rstd[:], Alu.mult,
                                   Alu.mult)

    ab = psm.tile([P, NQ], F32, tag="ab")
    nc.gpsimd.partition_broadcast(ab[:], rstd[:1, :])
    cb = psm.tile([P, NQ], F32, tag="cb")
    nc.gpsimd.partition_broadcast(cb[:], cc[:1, :])

    for c in range(DC):
        t1 = pg.tile([P, NQ], F32, tag="rot_f32", bufs=2)
        nc.vector.tensor_tensor(t1[:], h_f32[:, c, :], ab[:], Alu.mult)
        nc.vector.tensor_tensor(t1[:], t1[:], cb[:], Alu.add)
        if y_bf is not None:
            # y_bf (the next matmul input) on the short path: ACT applies the
            # per-feature affine; the f32 copy for the residual goes to the
            # otherwise-idle gpsimd engine
            nc.scalar.activation(y_bf[:, c, :], t1[:], Act.Identity,
                                 bias=(bq_c if bq_c is not None else b_c)[:, c:c + 1],
                                 scale=(gq_c if gq_c is not None else g_c)[:, c:c + 1])
            nc.gpsimd.tensor_scalar(y_f32[:, c, :], t1[:], g_c[:, c:c + 1],
                                    b_c[:, c:c + 1], Alu.mult, Alu.add)
        elif out_dram is not None:
            nc.scalar.activation(y_f32[:, c, :], t1[:], Act.Identity,
                                 bias=b_c[:, c:c + 1], scale=g_c[:, c:c + 1])
            nc.sync.dma_start(out_dram[:, c, :], y_f32[:, c, :])
        else:
            nc.vector.tensor_scalar(y_f32[:, c, :], t1[:], g_c[:, c:c + 1],
                                    b_c[:, c:c + 1], Alu.mult, Alu.add)


def build_nc(cfg):
    B, T, S, D, H, F = (cfg[k] for k in "BTSDHF")
    NQ = cfg["NQ"] = T // 2
    DC, TC, SC, FC = D // P, T // P, S // P, F // P
    HD = D // H

    nc = bacc.Bacc("TRN2", target_bir_lowering=False,
                   debug=cfg.get("debug", False), num_devices=2 * B)
    dp = nc.declare_dram_parameter
    xT_d = dp("xT", [D, T], E4, isOutput=False)
    xqT_d = dp("xqT", [D, NQ], E4, isOutput=False)
    xres_d = dp("xres", [D, NQ], F32, isOutput=False)
    encT_d = dp("encT", [D, S], E4, isOutput=False)
    mskT_d = dp("emT", [T, NQ], BF, isOutput=False) if cfg["self_mask"] else None
    emskT_d = dp("cemT", [S, NQ], BF, isOutput=False) if cfg["cross_mask"] else None
    w_d = {}
    for nm in ("sa_wq", "sa_wk", "ca_wq", "ca_wk"):
        w_d[nm] = dp(nm + "T", [DC, P, DC, P], E4, isOutput=False)
    for nm in ("sa_wo", "ca_wo"):
        w_d[nm] = dp(nm + "T", [DC, P, DC, P], BF, isOutput=False)
    for nm in ("sa_wv", "ca_wv"):
        w_d[nm] = dp(nm + "T", [D, D], E4, isOutput=False)
    f1_d = dp("f1T", [FC, P, DC, P], BF, isOutput=False)
    f2_d = dp("f2T", [DC, P, FC, P], BF, isOutput=False)
    blob_d = dp("cols_blob", [P, NB], F32, isOutput=False)
    bvrow_d = dp("sa_bv_row", [1, D], BF, isOutput=False)
    cvrow_d = dp("ca_bv_row", [1, D], BF, isOutput=False)
    outT_d = dp("outT", [D, NQ], F32, isOutput=True)

    with tile.TileContext(nc) as tc:
        with tc.tile_pool(name="const", bufs=1) as pc, \
             tc.tile_pool(name="glob", bufs=1) as pg, \
             tc.tile_pool(name="wpool", bufs=1) as pw, \
             tc.tile_pool(name="small", bufs=1) as psm, \
             tc.tile_pool(name="ps_proj", bufs=2, space="PSUM") as pps, \
             tc.tile_pool(name="ps_score", bufs=2, space="PSUM") as psa, \
             tc.tile_pool(name="ps_pv", bufs=2, space="PSUM") as ppv:

            # constants
            ones_bf = pc.tile([P, 1], BF)
            nc.vector.memset(ones_bf[:], 1.0 / cfg["D"])
            ones_f32 = pc.tile([1, P], F32)
            nc.vector.memset(ones_f32[:], 1.0)
            eps_t = pc.tile([1, 1], F32, tag="eps")
            nc.vector.memset(eps_t[:], float(cfg["eps"]))
            logsp = pc.tile([P, 1], F32, tag="logsp")
            nc.vector.memset(logsp[:], LOG_SP)
            blob_t = pc.tile([P, NB], F32, tag="blob")
            # gpsimd queue: keeps sync's first issues for wk0/kv0 (the
            # critical path to the first matmul); cols aren't needed until
            # the first dequant ~16us in
            nc.gpsimd.dma_start(blob_t[:], blob_d[:])
            bc_sb = {}
            off = 0
            for nm, w in COLS:
                bc_sb[nm] = blob_t[:, off:off + w]
                off += w
            bvrow_sb = pc.tile([1, D], BF, tag="bvrow_sa")
            nc.gpsimd.dma_start(bvrow_sb[:], bvrow_d[:])
            cvrow_sb = pc.tile([1, D], BF, tag="bvrow_ca")
            nc.gpsimd.dma_start(cvrow_sb[:], cvrow_d[:])

            pools = dict(w=pw, small=psm, ps_proj=pps, ps_score=psa,
                         ps_pv=ppv, ones_bf=ones_bf, ones_f32=ones_f32,
                         logsp=logsp, eps=eps_t)

            # globals: residual-chain f32 slots and q-source fp8 slots
            xq_sb = pg.tile([P, DC, NQ], E4, tag="qsrc8", bufs=1)
            nc.gpsimd.dma_start(xq_sb[:], xqT_d.rearrange("(c p) n -> p c n", p=P))
            xres_sb = pg.tile([P, DC, NQ], F32, tag="af32", bufs=2)
            nc.gpsimd.dma_start(xres_sb[:], xres_d.rearrange("(c p) n -> p c n", p=P))

            # cross-attention K-proj hoisted as background units: input-only
            # deps, run interleaved into the self-attention core phase where
            # the tensor engine otherwise stalls on pT/softmax dependencies
            enc_sb = pg.tile([P, DC, S], E4, tag="encsb", bufs=1)
            enc_r = encT_d.rearrange("(c p) n -> p c n", p=P)
            for c in range(DC):
                nc.gpsimd.dma_start(enc_sb[:, c, :], enc_r[:, c, :])
            cksb = pg.tile([P, DC, S], BF, tag="cksb", bufs=1)
            KBX = min(512, S)
            ck_hold = {}

            def _ck_step(od, nb, c):
                def run():
                    if nb == 0 and c == 0:
                        wk_t = pw.tile([P, DC, P], E4, tag="wod8", bufs=3)
                        nc.sync.dma_start(wk_t[:], w_d["ca_wk"][od])
                        ck_hold["wk"] = wk_t
                    if c == 0:
                        ps = pps.tile([P, KBX], F32, tag="proj")
                        ck_hold[(od, nb)] = ps
                    ps = ck_hold[(od, nb)]
                    nc.tensor.matmul(ps[:], ck_hold["wk"][:, c:c + 2, :],
                                     enc_sb[:, c:c + 2,
                                            nb * KBX:(nb + 1) * KBX],
                                     start=(c == 0), stop=(c == DC - 2),
                                     perf_mode=DR)
                return run

            def _ck_dq(od, nb):
                def run():
                    ps = ck_hold.pop((od, nb))
                    nc.vector.tensor_scalar(cksb[:, od, nb * KBX:(nb + 1) * KBX],
                                            ps[:], bc_sb["ca_dqk"][:, od:od + 1],
                                            bc_sb["ca_bk"][:, od:od + 1],
                                            Alu.mult, Alu.add)
                return run

            # 80 sub-unit closures consumed one per score-block/normalize slot
            ck_q = []
            for od in range(DC):
                for nb in range(S // KBX):
                    for c in range(0, DC, 2):
                        ck_q.append(_ck_step(od, nb, c))
                    ck_q.append(_ck_dq(od, nb))

            causal = cfg["self_mask"] and cfg.get("causal", False)
            sa_prm = dict(wq=w_d["sa_wq"], wk=w_d["sa_wk"], wv=w_d["sa_wv"],
                          wo=w_d["sa_wo"], bq=bc_sb["sa_bq"], bk=bc_sb["sa_bk"],
                          bo=bc_sb["sa_bo"], dqq=bc_sb["sa_dqq"],
                          dqk=bc_sb["sa_dqk"], dqv=bc_sb["sa_dqv"],
                          bv_row=bvrow_sb, fp8_pv=not cfg["self_mask"],
                          causal=causal, bg=ck_q)
            ca_prm = dict(wq=w_d["ca_wq"], wk=w_d["ca_wk"], wv=w_d["ca_wv"],
                          wo=w_d["ca_wo"], bq=bc_sb["ca_bq"], bk=bc_sb["ca_bk"],
                          bo=bc_sb["ca_bo"], dqq=bc_sb["ca_dqq"],
                          dqk=bc_sb["ca_dqk"], dqv=bc_sb["ca_dqv"],
                          bv_row=cvrow_sb, fp8_pv=not cfg["cross_mask"],
                          pre_kv=enc_sb, pre_ksb=cksb)

            # ---- self attention + LN1 ----
            h1 = pg.tile([P, DC, NQ], F32, tag="af32", bufs=2)
            cfg["_dbg_pfx"] = "sa"
            with tc.tile_pool(name="attn1", bufs=1) as pa:
                hbf, sq = _attention(nc, pa, pools, cfg, xT_d, T, xq_sb, mskT_d,
                                     xres_sb, sa_prm, h1, pg)
                y1 = pg.tile([P, DC, NQ], F32, tag="af32", bufs=2)
                y1b = pg.tile([P, DC, NQ], E4, tag="qsrc8", bufs=1)
                _layernorm(nc, pg, pools, cfg, h1, hbf, sq,
                           bc_sb["ln1_g"], bc_sb["ln1_b"], y1, y1b,
                           gq_c=bc_sb["ln1_gq"], bq_c=bc_sb["ln1_bq"])

            # ---- cross attention + LN2 ----
            h2 = pg.tile([P, DC, NQ], F32, tag="af32", bufs=2)
            cfg["_dbg_pfx"] = "ca"
            with tc.tile_pool(name="attn2", bufs=1) as pa:
                hbf, sq = _attention(nc, pa, pools, cfg, encT_d, S, y1b, emskT_d,
                                     y1, ca_prm, h2, pg)
                y2 = pg.tile([P, DC, NQ], F32, tag="af32", bufs=2)
                y2b = pg.tile([P, DC, NQ], BF, tag="qsrc", bufs=1)
                _layernorm(nc, pg, pools, cfg, h2, hbf, sq,
                           bc_sb["ln2_g"], bc_sb["ln2_b"], y2, y2b)

            if cfg.get("dbg"):
                for nm, t in (("h1", h1), ("y1", y1), ("h2", h2), ("y2", y2)):
                    d = dp(f"dbg_{nm}", list(t.shape), t.dtype, isOutput=True)
                    nc.sync.dma_start(d[:], t[:])

            # ---- FFN + LN3 ----
            with tc.tile_pool(name="ffn", bufs=1) as pa:
                fsb = pa.tile([P, FC, NQ], BF, tag="fsb")
                for ft in range(FC):
                    w1 = pw.tile([P, DC, P], BF, tag="wod", bufs=3)
                    nc.sync.dma_start(w1[:], f1_d[ft])
                    ps = pps.tile([P, NQ], F32, tag="proj")
                    for c in range(DC):
                        nc.tensor.matmul(ps[:], w1[:, c, :], y2b[:, c, :],
                                         start=(c == 0), stop=(c == DC - 1))
                    nc.scalar.activation(fsb[:, ft, :], ps[:], cfg["gelu"],
                                         bias=bc_sb["fc1_b"][:, ft:ft + 1])
                h3 = pg.tile([P, DC, NQ], F32, tag="af32", bufs=2)
                hbf, sq = [], []
                for od in range(DC):
                    w2 = pa.tile([P, FC, P], BF, tag="w2", bufs=2)
                    nc.sync.dma_start(w2[:], f2_d[od])
                    ps = pps.tile([P, NQ], F32, tag="proj")
                    for fc_ in range(FC):
                        nc.tensor.matmul(ps[:], w2[:, fc_, :], fsb[:, fc_, :],
                                         start=(fc_ == 0), stop=(fc_ == FC - 1))
                    nc.vector.scalar_tensor_tensor(h3[:, od, :], ps[:],
                                                   bc_sb["fc2_b"][:, od:od + 1],
                                                   y2[:, od, :], Alu.add, Alu.add)
                    hb = pg.tile([P, NQ], BF, tag="rot_hbf", bufs=2)
                    nc.scalar.copy(hb[:], h3[:, od, :])
                    s2 = pg.tile([P, NQ], BF, tag="rot_sq", bufs=2)
                    nc.vector.tensor_tensor(s2[:], hb[:], hb[:], Alu.mult)
                    hbf.append(hb)
                    sq.append(s2)
                out_f = pg.tile([P, DC, NQ], F32, tag="af32", bufs=2)
                _layernorm(nc, pg, pools, cfg, h3, hbf, sq,
                           bc_sb["ln3_g"], bc_sb["ln3_b"], out_f, None,
                           out_dram=outT_d.rearrange("(c p) n -> p c n", p=P))

    nc.compile()
    return nc


def make_in_maps(cfg, inputs):
    B, T, S, D, H, F = (cfg[k] for k in "BTSDHF")
    NQ = T // 2
    DC, FC = D // P, F // P
    HD = D // H
    bf = ml_dtypes.bfloat16

    def col(v):  # [D'] -> [P, D'//P]
        return np.ascontiguousarray(np.asarray(v, np.float32).reshape(-1, P).T)

    def wtile(w):  # [DO, DI] -> [DO/P, P, DI/P, P] od-tiles of transposed weight
        w = np.asarray(w, np.float32)
        do, di = w.shape
        return np.ascontiguousarray(
            w.reshape(do // P, P, di // P, P).transpose(0, 3, 2, 1)).astype(bf)

    def wtile8(w):  # fp8 od-tiles + per-output-row dequant (absmax/240)
        w = np.asarray(w, np.float32)
        do, di = w.shape
        am = np.abs(w).max(axis=1)
        s = 240.0 / np.maximum(am, 1e-30)
        w8 = (w * s[:, None]).astype(E4np)
        t = np.ascontiguousarray(
            w8.reshape(do // P, P, di // P, P).transpose(0, 3, 2, 1))
        return t, (1.0 / s).astype(np.float32)

    x_np = np.asarray(inputs["hidden_states"], np.float32)
    enc_np = np.asarray(inputs["encoder_hidden_states"], np.float32)
    s_x = 240.0 / max(float(np.abs(x_np).max()), 1e-30)
    s_enc = 240.0 / max(float(np.abs(enc_np).max()), 1e-30)

    shared = {}
    sc = HD ** -0.5
    t, dq = wtile8(np.asarray(inputs["sa_wq"]) * sc)
    cols = {}
    shared["sa_wqT"] = t
    cols["sa_dqq"] = col(dq / s_x)
    t, dq = wtile8(np.asarray(inputs["ca_wq"]) * sc)
    shared["ca_wqT"] = t
    cols["ca_dqq"] = col(dq / SY)
    t, dq = wtile8(inputs["sa_wk"])
    shared["sa_wkT"] = t
    cols["sa_dqk"] = col(dq / s_x)
    t, dq = wtile8(inputs["ca_wk"])
    shared["ca_wkT"] = t
    cols["ca_dqk"] = col(dq / s_enc)
    for nm in ("sa_wo", "ca_wo"):
        shared[nm + "T"] = wtile(inputs[nm])
    # V weights: per-tensor scale (dequant rides the free axis -> one scalar)
    wv = np.asarray(inputs["sa_wv"], np.float32)
    s_wv = 240.0 / max(float(np.abs(wv).max()), 1e-30)
    shared["sa_wvT"] = np.ascontiguousarray((wv.T * s_wv)).astype(E4np)
    cols["sa_dqv"] = np.full((P, 1), 1.0 / (s_wv * s_x), np.float32)
    wv = np.asarray(inputs["ca_wv"], np.float32)
    s_wv = 240.0 / max(float(np.abs(wv).max()), 1e-30)
    shared["ca_wvT"] = np.ascontiguousarray((wv.T * s_wv)).astype(E4np)
    cols["ca_dqv"] = np.full((P, 1), SV / (s_wv * s_enc), np.float32)
    shared["f1T"] = wtile(inputs["fc1_w"])
    shared["f2T"] = wtile(inputs["fc2_w"])
    cols["sa_bq"] = col(np.asarray(inputs["sa_bq"], np.float32) * sc)
    cols["ca_bq"] = col(np.asarray(inputs["ca_bq"], np.float32) * sc)
    for nm in ("sa_bk", "sa_bo", "ca_bk", "ca_bo", "fc2_b", "fc1_b",
               "ln1_g", "ln1_b", "ln2_g", "ln2_b", "ln3_g", "ln3_b"):
        cols[nm] = col(inputs[nm])
    cols["ln1_gq"] = col(np.asarray(inputs["ln1_g"], np.float32) * SY)
    cols["ln1_bq"] = col(np.asarray(inputs["ln1_b"], np.float32) * SY)
    shared["cols_blob"] = np.ascontiguousarray(
        np.concatenate([cols[nm] for nm, _ in COLS], axis=1))
    shared["sa_bv_row"] = np.asarray(inputs["sa_bv"], np.float32)[None, :].astype(bf)
    shared["ca_bv_row"] = (np.asarray(inputs["ca_bv"], np.float32)[None, :]
                           * SV).astype(bf)

    causal = cfg["self_mask"] and cfg.get("causal", False)
    in_maps = []
    for c in range(2 * B):
        b, half = divmod(c, 2)
        if causal:
            qs = np.concatenate([np.arange(blk * P, (blk + 1) * P)
                                 for blk in STRIPES[half]])
        else:
            qs = np.arange(half * NQ, (half + 1) * NQ)
        x = x_np[b]  # [T, D]
        m = {}
        m.update(shared)
        m["xT"] = np.ascontiguousarray((x.T * s_x)).astype(E4np)
        m["xqT"] = np.ascontiguousarray((x[qs].T * s_x)).astype(E4np)
        m["xres"] = np.ascontiguousarray(x[qs].T)
        m["encT"] = np.ascontiguousarray((enc_np[b].T * s_enc)).astype(E4np)
        if cfg.get("self_mask", True):
            m["emT"] = np.ascontiguousarray(np.exp(
                np.asarray(inputs["attention_mask"][b, 0], np.float32)[qs].T)).astype(bf)
        if cfg.get("cross_mask", False):
            m["cemT"] = np.ascontiguousarray(np.exp(
                np.asarray(inputs["encoder_attention_mask"][b, 0], np.float32)[qs].T)).astype(bf)
        in_maps.append(m)
    return in_maps


_NC_CACHE = {}


def get_nc(cfg=None):
    cfg = cfg or default_cfg()
    key = tuple(sorted((k, str(v)) for k, v in cfg.items()))
    if key not in _NC_CACHE:
        _NC_CACHE[key] = build_nc(dict(cfg))
    return _NC_CACHE[key]


def _is_causal_mask(mask, T):
    m = np.asarray(mask)
    tri = np.arange(T)[:, None] >= np.arange(T)[None, :]
    return bool(np.all((m[:, 0] > -1.0) == tri))


def kernel(**inputs):
    from concourse.bass_utils import run_bass_kernel_spmd

    cfg = default_cfg()
    cfg["self_mask"] = bool(np.any(np.asarray(inputs["attention_mask"])))
    cfg["cross_mask"] = bool(np.any(np.asarray(inputs["encoder_attention_mask"])))
    B, T, D = cfg["B"], cfg["T"], cfg["D"]
    NQ = T // 2
    cfg["causal"] = (cfg["self_mask"]
                     and _is_causal_mask(inputs["attention_mask"], T))
    causal = cfg["self_mask"] and cfg["causal"]
    nc = get_nc(cfg)
    in_maps = make_in_maps(cfg, inputs)
    res = run_bass_kernel_spmd(nc, in_maps, list(range(2 * B))).results
    out = np.empty((B, T, D), np.float32)
    for c in range(2 * B):
        b, half = divmod(c, 2)
        if causal:
            qi = np.concatenate([np.arange(blk * P, (blk + 1) * P)
                                 for blk in STRIPES[half]])
            out[b, qi, :] = res[c]["outT"].T
        else:
            out[b, half * NQ:(half + 1) * NQ, :] = res[c]["outT"].T
    return out


# revision 51
# speedup vs baseline: 1.0160x; 1.0042x over previous
"""BART decoder layer on 8 TRN2 NeuronCores.

Sharding: data-parallel over (batch, query-half): core c handles batch c//2,
query rows [half*512, half*512+512). Each core computes the full decoder layer
for its 512 query tokens; self/cross K,V are recomputed per core from the full
batch sequence (no collectives).

On-device layout is "transposed": activations live as [feature, token] so every
matmul contracts along the SBUF partition axis. Q/K/V projections run in fp8
(e4m3) with MatmulPerfMode.DoubleRow (2 reduction chunks per pass, 2x rate);
weights are host-quantized per-output-row, activations per-tensor, and dequant
scales fold into the existing bias-add ops. Cross-attention P*V also runs fp8
DoubleRow: exp() writes fp8 pT directly (output scale folded into the exp bias)
and the softmax normalization cancels both the P and V scales. Self-attention
scores/PV and out-proj/FFN stay bf16 (error budget). Accumulation is f32 in
PSUM, residuals/LayerNorm are f32. Softmax skips max-subtraction; row sums come
from an extra ones-column appended to V. LayerNorm partition-axis sums use
ones-matmuls on the TensorEngine.
"""

import sys

sys.path.insert(0, "/opt/trn_rl_repo")

import ml_dtypes
import numpy as np

import concourse.bacc as bacc
import concourse.bass as bass
import concourse.mybir as mybir
import concourse.tile as tile

BF = mybir.dt.bfloat16
F32 = mybir.dt.float32
E4 = mybir.dt.float8e4
E4np = ml_dtypes.float8_e4m3
P = 128
Act = mybir.ActivationFunctionType
Alu = mybir.AluOpType
DR = mybir.MatmulPerfMode.DoubleRow

SP = 2.0    # cross pT storage scale: pT = exp(score) * SP (scores stay < ~4.8)
SV = 16.0   # cross vsb storage scale: vsb = V * SV
SY = 16.0   # y1b (cross-attn q source) storage scale
LOG_SP = float(np.log(SP))

# causal query striping: blocks {0,3,4,7}/{1,2,5,6} per core half; columns
# [0,256) then only ever need keys [0,512) (4 k-chunks), columns [256,512)
# need all 8 -- the skip pattern is the same static program on every core
STRIPES = ([0, 3, 4, 7], [1, 2, 5, 6])

# const-column blob layout: one DMA instead of ~22 serialized descriptor issues
COLS = [("sa_bq", 8), ("sa_bk", 8), ("sa_bo", 8), ("ca_bq", 8), ("ca_bk", 8),
        ("ca_bo", 8), ("fc2_b", 8), ("ln1_g", 8), ("ln1_b", 8), ("ln1_gq", 8),
        ("ln1_bq", 8), ("ln2_g", 8), ("ln2_b", 8), ("ln3_g", 8), ("ln3_b", 8),
        ("sa_dqq", 8), ("sa_dqk", 8), ("ca_dqq", 8), ("ca_dqk", 8),
        ("sa_dqv", 1), ("ca_dqv", 1), ("fc1_b", 32)]
NB = sum(w for _, w in COLS)


def default_cfg():
    return dict(B=4, T=1024, S=1024, D=1024, H=16, F=4096, eps=1e-5,
                gelu=Act.Gelu, self_mask=True, cross_mask=False, causal=True)


def _attention(nc, pa, pools, cfg, kv_dram, L, q_sb, msk_dram, res_sb,
               prm, h_f32, pg):
    """One multi-head attention block, fully in transposed layout.

    pa: phase-scoped SBUF pool.
    kv_dram: [D, L] fp8 dram AP (source tokens for K/V)
    q_sb:    [P, DC, NQ] fp8 sbuf (source for Q)
    msk_dram:[L, NQ] bf16 dram AP of exp(mask) factors, or None (no masking)
    res_sb:  [P, DC, NQ] f32 sbuf (residual)
    prm: dict with weight dram APs (wq/wk fp8 od-tiles, wv fp8 [D,D], wo bf16
         od-tiles), bias cols, dequant cols (dqq/dqk [P,DC], dqv [P,1]) and
         fp8_pv flag. When fp8_pv: pT/vsb are fp8 and PV runs DoubleRow.
    h_f32:   [P, DC, NQ] f32 sbuf out (attn_out + bias + residual)
    Returns (h_bf_tiles, sq_tiles) lists used by LayerNorm stats.
    """
    D, H, NQ = cfg["D"], cfg["H"], cfg["NQ"]
    HD = D // H
    DC, LC = D // P, L // P
    HPC = P // HD  # heads per 128-row chunk
    KB = min(512, L)  # K-proj column block
    VB = min(512, D)  # V-proj column block
    pw, pps, psa, ppv, psm = (pools[k] for k in
                              ("w", "ps_proj", "ps_score", "ps_pv", "small"))
    fp8_pv = prm["fp8_pv"]
    causal = prm.get("causal", False)
    NQH = NQ // 2
    wq_d, wk_d, wv_d, wo_d = prm["wq"], prm["wk"], prm["wv"], prm["wo"]
    bq_c, bk_c, bo_c = prm["bq"], prm["bk"], prm["bo"]
    dqq_c, dqk_c, dqv_c = prm["dqq"], prm["dqk"], prm["dqv"]
    bg = list(prm.get("bg") or [])

    if prm.get("pre_kv") is not None:
        kv_sb = prm["pre_kv"]
    else:
        # chunked load so the first K-proj matmul starts after chunk 0 lands
        wk_first = pw.tile([P, DC, P], E4, tag="wod8", bufs=3)
        nc.sync.dma_start(wk_first[:], wk_d[0])
        kv_sb = pa.tile([P, DC, L], E4, tag="kvsrc")
        kv_r = kv_dram.rearrange("(c p) n -> p c n", p=P)
        for c in range(DC):
            eng = nc.sync if c % 2 == 0 else nc.scalar
            eng.dma_start(kv_sb[:, c, :], kv_r[:, c, :])
    if msk_dram is not None:
        msk_sb = pa.tile([P, LC, NQ], BF, tag="msk")
        nc.gpsimd.dma_start(msk_sb[:], msk_dram.rearrange("(c p) n -> p c n", p=P))

    # V bias broadcast row -> [P, D] (pre-scaled by SV when fp8_pv)
    vbias = pa.tile([P, D], BF, tag="vbias")
    nc.gpsimd.partition_broadcast(vbias[:], prm["bv_row"][:1, :])

    # K^T [d_out, k]  (ksb[p, od, k]); fp8 DoubleRow over chunk pairs
    if prm.get("pre_ksb") is not None:
        ksb = prm["pre_ksb"]
    else:
        ksb = pa.tile([P, DC, L], BF, tag="ksb")
        for od in range(DC):
            if od == 0:
                wk_t = wk_first
            else:
                wk_t = pw.tile([P, DC, P], E4, tag="wod8", bufs=3)
                nc.sync.dma_start(wk_t[:], wk_d[od])
            for nb in range(L // KB):
                ps = pps.tile([P, KB], F32, tag="proj")
                for c in range(0, DC, 2):
                    nc.tensor.matmul(ps[:], wk_t[:, c:c + 2, :],
                                     kv_sb[:, c:c + 2, nb * KB:(nb + 1) * KB],
                                     start=(c == 0), stop=(c == DC - 2),
                                     perf_mode=DR)
                nc.vector.tensor_scalar(ksb[:, od, nb * KB:(nb + 1) * KB],
                                        ps[:], dqk_c[:, od:od + 1],
                                        bk_c[:, od:od + 1], Alu.mult, Alu.add)

    # V natural layout + ones column: vsb[p, t, h*65:(h+1)*65], col 64 = SV*1
    wv_sb = pw.tile([P, DC, D], E4, tag="wvfull", bufs=1)
    nc.gpsimd.dma_start(wv_sb[:], wv_d.rearrange("(c p) n -> p c n", p=P))
    vdt = E4 if fp8_pv else BF
    vsb = pa.tile([P, LC, H * (HD + 1)], vdt, tag="vsb")
    vsb_r = vsb[:].rearrange("p t (h w) -> p t h w", w=HD + 1)
    for tt in range(LC):
        nc.vector.memset(vsb_r[:, tt, :, HD:HD + 1], SV if fp8_pv else 1.0)
        for nb in range(D // VB):
            ps = pps.tile([P, VB], F32, tag="proj")
            for c in range(0, DC, 2):
                nc.tensor.matmul(ps[:], kv_sb[:, c:c + 2, tt * P:(tt + 1) * P],
                                 wv_sb[:, c:c + 2, nb * VB:(nb + 1) * VB],
                                 start=(c == 0), stop=(c == DC - 2),
                                 perf_mode=DR)
            hpb = VB // HD  # heads per block
            nc.vector.scalar_tensor_tensor(
                vsb_r[:, tt, nb * hpb:(nb + 1) * hpb, :HD],
                ps[:].rearrange("p (h w) -> p h w", w=HD),
                dqv_c[:, 0:1],
                vbias[:, nb * VB:(nb + 1) * VB].rearrange("p (h w) -> p h w", w=HD),
                Alu.mult, Alu.add)

    # Q^T [d_out, q]; fp8 DoubleRow
    qsb = pa.tile([P, DC, NQ], BF, tag="qsb")
    for od in range(DC):
        wq_t = pw.tile([P, DC, P], E4, tag="wod8", bufs=3)
        nc.sync.dma_start(wq_t[:], wq_d[od])
        ps = pps.tile([P, NQ], F32, tag="proj")
        for c in range(0, DC, 2):
            nc.tensor.matmul(ps[:], wq_t[:, c:c + 2, :], q_sb[:, c:c + 2, :],
                             start=(c == 0), stop=(c == DC - 2), perf_mode=DR)
        nc.vector.tensor_scalar(qsb[:, od, :], ps[:], dqq_c[:, od:od + 1],
                                bq_c[:, od:od + 1], Alu.mult, Alu.add)

    # attention per head -> osb [d, q] bf16 (unnormalized; batch-normalized
    # below). With causal striping, scores/exp for kc>=LC/2 only cover query
    # columns [NQH,NQ); the untouched pT quadrant is zeroed once per buffer so
    # the full-range PV accumulation reads zeros there.
    osb = pa.tile([P, DC, NQ], BF, tag="osb")
    # compact pT: with causal, kc>=LC/2 rows only hold query columns
    # [NQH,NQ) -- the dead quadrant is never stored, and PV accumulates the
    # right half only for those chunks (first matmul zeroes the whole bank)
    if causal:
        ptw = (LC // 2) * NQ + (LC // 2) * NQH
    else:
        ptw = LC * NQ

    def pt_off(kc):
        if not causal or kc < LC // 2:
            return kc * NQ
        return (LC // 2) * NQ + (kc - LC // 2) * NQH

    pt_a = pa.tile([P, ptw], E4 if fp8_pv else BF, tag="pT0")
    pt_b = pa.tile([P, ptw], E4 if fp8_pv else BF, tag="pT1")
    pt_tiles = [pt_a, pt_b]
    for h in range(H):
        od, po = divmod(h, HPC)
        po *= HD
        pT = pt_tiles[h % 2]
        for kb in range(LC // 2):
            # two k-tiles share one 2-bank PSUM tile so exp / mask-mult run as
            # one double-width op (halves per-op fixed overhead on ACT/DVE)
            half = causal and kb >= LC // 4
            cs = slice(NQH, NQ) if half else slice(0, NQ)
            w = NQH if half else NQ
            ps_s = psa.tile([P, 2, w], F32, tag="score")
            for j in range(2):
                kc = kb * 2 + j
                nc.tensor.matmul(ps_s[:, j, :],
                                 ksb[po:po + HD, od, kc * P:(kc + 1) * P],
                                 qsb[po:po + HD, od, cs], start=True, stop=True)
            off = pt_off(kb * 2)
            ptv = pT[:, off:off + 2 * w].rearrange("p (j n) -> p j n", j=2)
            if msk_dram is not None:
                sc = pa.tile([P, 2, w], BF, tag="sc", bufs=2)
                nc.scalar.activation(sc[:], ps_s[:], Act.Exp)
                nc.vector.tensor_tensor(ptv[:, :, :], sc[:],
                                        msk_sb[:, kb * 2:(kb + 1) * 2, cs],
                                        Alu.mult)
            else:
                # fp8 pT: output scale SP folded into the exp bias
                nc.scalar.activation(ptv[:, :, :], ps_s[:], Act.Exp,
                                     bias=pools["logsp"][:, 0:1] if fp8_pv
                                     else 0.0)
            if bg:
                bg.pop(0)()
        ps_o = ppv.tile([HD + 1, NQ], F32, tag="pv")
        if fp8_pv:
            # fp8 DoubleRow over k-chunk pairs; SP*SV cancels in normalization
            for kc in range(0, LC, 2):
                ptv = pT[:, kc * NQ:(kc + 2) * NQ].rearrange(
                    "p (j n) -> p j n", j=2)
                nc.tensor.matmul(ps_o[:],
                                 vsb[:, kc:kc + 2, h * (HD + 1):(h + 1) * (HD + 1)],
                                 ptv[:, :, :],
                                 start=(kc == 0), stop=(kc == LC - 2),
                                 perf_mode=DR)
        else:
            for kc in range(LC):
                off = pt_off(kc)
                if causal and kc >= LC // 2:
                    nc.tensor.matmul(ps_o[:, NQH:],
                                     vsb[:, kc, h * (HD + 1):(h + 1) * (HD + 1)],
                                     pT[:, off:off + NQH], start=False,
                                     stop=(kc == LC - 1),
                                     skip_group_check=True)
                else:
                    nc.tensor.matmul(ps_o[:],
                                     vsb[:, kc, h * (HD + 1):(h + 1) * (HD + 1)],
                                     pT[:, off:off + NQ], start=(kc == 0),
                                     stop=(not causal and kc == LC - 1),
                                     skip_group_check=causal)
        rl = psm.tile([1, NQ], F32, tag="rl", bufs=2)
        if cfg.get("recip_fast", True):
            # custom-DVE ops misread PSUM at partition offset 64; stage the
            # denominator row to a partition-0 SBUF tile first
            rls = psm.tile([1, NQ], F32, tag="rls", bufs=2)
            # standard DVE op: safe on PSUM@64 (only custom-DVE ops misread);
            # keeps the staging copy off the exp-saturated ACT engine
            nc.vector.tensor_copy(rls[:], ps_o[HD:HD + 1, :])
            nc.vector.reciprocal_approx_fast(rl[:], rls[:])
        else:
            nc.vector.reciprocal(rl[:], ps_o[HD:HD + 1, :])
        rb = psm.tile([HD, NQ], F32, tag="rb", bufs=2)
        nc.gpsimd.partition_broadcast(rb[:], rl[:1, :])
        nc.vector.tensor_tensor(osb[po:po + HD, od, :], ps_o[:HD, :], rb[:],
                                Alu.mult)
        # background cross-K sub-units (single matmuls) fill this phase's
        # sub-microsecond tensor stall cycles
        if bg:
            bg.pop(0)()
    while bg:
        bg.pop(0)()

    if cfg.get("dbg"):
        pfx = cfg["_dbg_pfx"]
        for nm, t in (("ksb", ksb), ("vsb", vsb), ("qsb", qsb), ("osb", osb)):
            d = nc.declare_dram_parameter(f"dbg_{pfx}_{nm}", list(t.shape),
                                          t.dtype, isOutput=True)
            nc.sync.dma_start(d[:], t[:])

    # out-proj + bias + residual -> h_f32 (f32); emit bf16 + square tiles for LN
    h_bf_tiles, sq_tiles = [], []
    for od in range(DC):
        wo_t = pw.tile([P, DC, P], BF, tag="wod", bufs=3)
        nc.sync.dma_start(wo_t[:], wo_d[od])
        ps = pps.tile([P, NQ], F32, tag="proj")
        for c in range(DC):
            nc.tensor.matmul(ps[:], wo_t[:, c, :], osb[:, c, :],
                             start=(c == 0), stop=(c == DC - 1))
        nc.vector.scalar_tensor_tensor(h_f32[:, od, :], ps[:], bo_c[:, od:od + 1],
                                       res_sb[:, od, :], Alu.add, Alu.add)
        hb = pg.tile([P, NQ], BF, tag="rot_hbf", bufs=2)
        nc.scalar.copy(hb[:], h_f32[:, od, :])
        sq = pg.tile([P, NQ], BF, tag="rot_sq", bufs=2)
        nc.vector.tensor_tensor(sq[:], hb[:], hb[:], Alu.mult)
        h_bf_tiles.append(hb)
        sq_tiles.append(sq)
    return h_bf_tiles, sq_tiles


def _layernorm(nc, pg, pools, cfg, h_f32, h_bf_tiles, sq_tiles, g_c, b_c,
               y_f32, y_bf, gq_c=None, bq_c=None, out_dram=None):
    """y = LN(h) over the partition (feature) axis via ones-matmul stats.

    y_bf (if set) is the next matmul input; when gq_c/bq_c are given, y_bf is
    fp8 and they carry the extra output scale. out_dram (if set) receives
    y_f32 chunk-by-chunk so the final store overlaps the normalize loop.
    """
    D, NQ, eps = cfg["D"], cfg["NQ"], cfg["eps"]
    DC = D // P
    psm, pstat = pools["small"], pools["ps_score"]
    ones_bf = pools["ones_bf"]

    ps_sx = pstat.tile([1, NQ], F32, tag="score")
    for c in range(DC):
        nc.tensor.matmul(ps_sx[:], ones_bf[:], h_bf_tiles[c][:],
                         start=(c == 0), stop=(c == DC - 1))
    ps_sq = pstat.tile([1, NQ], F32, tag="score")
    for c in range(DC):
        nc.tensor.matmul(ps_sq[:], ones_bf[:], sq_tiles[c][:],
                         start=(c == 0), stop=(c == DC - 1))

    # ones_bf carries 1/D: ps_sx = mean, ps_sq = mean-of-squares
    m2 = psm.tile([1, NQ], F32, tag="m2")
    nc.scalar.square(m2[:], ps_sx[:])
    var = psm.tile([1, NQ], F32, tag="var")
    nc.vector.tensor_sub(var[:], ps_sq[:], m2[:])
    sd = psm.tile([1, NQ], F32, tag="sd")
    nc.scalar.activation(sd[:], var[:], Act.Sqrt, bias=pools["eps"][:, 0:1])
    rstd = psm.tile([1, NQ], F32, tag="rstd")
    nc.vector.reciprocal_approx_fast(rstd[:], sd[:])
    cc = psm.tile([1, NQ], F32, tag="cc")
    nc.vector.scalar_tensor_tensor(cc[:], ps_sx[:], -1.0, rstd[:], Alu.mult,
                                   Alu.mult)

    ab = psm.tile([P, NQ], F32, tag="ab")
    nc.gpsimd.partition_broadcast(ab[:], rstd[:1, :])
    cb = psm.tile([P, NQ], F32, tag="cb")
    nc.gpsimd.partition_broadcast(cb[:], cc[:1, :])

    for c in range(DC):
        t1 = pg.tile([P, NQ], F32, tag="rot_f32", bufs=2)
        nc.vector.tensor_tensor(t1[:], h_f32[:, c, :], ab[:], Alu.mult)
        nc.vector.tensor_tensor(t1[:], t1[:], cb[:], Alu.add)
        if y_bf is not None:
            # y_bf (the next matmul input) on the short path: ACT applies the
            # per-feature affine; the f32 copy for the residual goes to the
            # otherwise-idle gpsimd engine
            nc.scalar.activation(y_bf[:, c, :], t1[:], Act.Identity,
                                 bias=(bq_c if bq_c is not None else b_c)[:, c:c + 1],
                                 scale=(gq_c if gq_c is not None else g_c)[:, c:c + 1])
            nc.gpsimd.tensor_scalar(y_f32[:, c, :], t1[:], g_c[:, c:c + 1],
                                    b_c[:, c:c + 1], Alu.mult, Alu.add)
        elif out_dram is not None:
            nc.scalar.activation(y_f32[:, c, :], t1[:], Act.Identity,
                                 bias=b_c[:, c:c + 1], scale=g_c[:, c:c + 1])
            nc.sync.dma_start(out_dram[:, c, :], y_f32[:, c, :])
        else:
            nc.vector.tensor_scalar(y_f32[:, c, :], t1[:], g_c[:, c:c + 1],
                                    b_c[:, c:c + 1], Alu.mult, Alu.add)


def build_nc(cfg):
    B, T, S, D, H, F = (cfg[k] for k in "BTSDHF")
    NQ = cfg["NQ"] = T // 2
    DC, TC, SC, FC = D // P, T // P, S // P, F // P
    HD = D // H

    nc = bacc.Bacc("TRN2", target_bir_lowering=False,
                   debug=cfg.get("debug", False), num_devices=2 * B)
    dp = nc.declare_dram_parameter
    xT_d = dp("xT", [D, T], E4, isOutput=False)
    xqT_d = dp("xqT", [D, NQ], E4, isOutput=False)
    xres_d = dp("xres", [D, NQ], F32, isOutput=False)
    encT_d = dp("encT", [D, S], E4, isOutput=False)
    mskT_d = dp("emT", [T, NQ], BF, isOutput=False) if cfg["self_mask"] else None
    emskT_d = dp("cemT", [S, NQ], BF, isOutput=False) if cfg["cross_mask"] else None
    w_d = {}
    for nm in ("sa_wq", "sa_wk", "ca_wq", "ca_wk"):
        w_d[nm] = dp(nm + "T", [DC, P, DC, P], E4, isOutput=False)
    for nm in ("sa_wo", "ca_wo"):
        w_d[nm] = dp(nm + "T", [DC, P, DC, P], BF, isOutput=False)
    for nm in ("sa_wv", "ca_wv"):
        w_d[nm] = dp(nm + "T", [D, D], E4, isOutput=False)
    f1_d = dp("f1T", [FC, P, DC, P], BF, isOutput=False)
    f2_d = dp("f2T", [DC, P, FC, P], BF, isOutput=False)
    blob_d = dp("cols_blob", [P, NB], F32, isOutput=False)
    bvrow_d = dp("sa_bv_row", [1, D], BF, isOutput=False)
    cvrow_d = dp("ca_bv_row", [1, D], BF, isOutput=False)
    outT_d = dp("outT", [D, NQ], F32, isOutput=True)

    with tile.TileContext(nc) as tc:
        with tc.tile_pool(name="const", bufs=1) as pc, \
             tc.tile_pool(name="glob", bufs=1) as pg, \
             tc.tile_pool(name="wpool", bufs=1) as pw, \
             tc.tile_pool(name="small", bufs=1) as psm, \
             tc.tile_pool(name="ps_proj", bufs=2, space="PSUM") as pps, \
             tc.tile_pool(name="ps_score", bufs=2, space="PSUM") as psa, \
             tc.tile_pool(name="ps_pv", bufs=2, space="PSUM") as ppv:

            # constants
            ones_bf = pc.tile([P, 1], BF)
            nc.vector.memset(ones_bf[:], 1.0 / cfg["D"])
            ones_f32 = pc.tile([1, P], F32)
            nc.vector.memset(ones_f32[:], 1.0)
            eps_t = pc.tile([1, 1], F32, tag="eps")
            nc.vector.memset(eps_t[:], float(cfg["eps"]))
            logsp = pc.tile([P, 1], F32, tag="logsp")
            nc.vector.memset(logsp[:], LOG_SP)
            blob_t = pc.tile([P, NB], F32, tag="blob")
            # gpsimd queue: keeps sync's first issues for wk0/kv0 (the
            # critical path to the first matmul); cols aren't needed until
            # the first dequant ~16us in
            nc.gpsimd.dma_start(blob_t[:], blob_d[:])
            bc_sb = {}
            off = 0
            for nm, w in COLS:
                bc_sb[nm] = blob_t[:, off:off + w]
                off += w
            bvrow_sb = pc.tile([1, D], BF, tag="bvrow_sa")
            nc.gpsimd.dma_start(bvrow_sb[:], bvrow_d[:])
            cvrow_sb = pc.tile([1, D], BF, tag="bvrow_ca")
            nc.gpsimd.dma_start(cvrow_sb[:], cvrow_d[:])

            pools = dict(w=pw, small=psm, ps_proj=pps, ps_score=psa,
                         ps_pv=ppv, ones_bf=ones_bf, ones_f32=ones_f32,
                         logsp=logsp, eps=eps_t)

            # globals: residual-chain f32 slots and q-source fp8 slots
            xq_sb = pg.tile([P, DC, NQ], E4, tag="qsrc8", bufs=1)
            nc.gpsimd.dma_start(xq_sb[:], xqT_d.rearrange("(c p) n -> p c n", p=P))
            xres_sb = pg.tile([P, DC, NQ], F32, tag="af32", bufs=2)
            nc.gpsimd.dma_start(xres_sb[:], xres_d.rearrange("(c p) n -> p c n", p=P))

            # cross-attention K-proj hoisted as background units: input-only
            # deps, run interleaved into the self-attention core phase where
            # the tensor engine otherwise stalls on pT/softmax dependencies
            enc_sb = pg.tile([P, DC, S], E4, tag="encsb", bufs=1)
            enc_r = encT_d.rearrange("(c p) n -> p c n", p=P)
            for c in range(DC):
                nc.gpsimd.dma_start(enc_sb[:, c, :], enc_r[:, c, :])
            cksb = pg.tile([P, DC, S], BF, tag="cksb", bufs=1)
            KBX = min(512, S)
            ck_hold = {}

            def _ck_step(od, nb, c):
                def run():
                    if nb == 0 and c == 0:
                        wk_t = pw.tile([P, DC, P], E4, tag="wod8", bufs=3)
                        nc.sync.dma_start(wk_t[:], w_d["ca_wk"][od])
                        ck_hold["wk"] = wk_t
                    if c == 0:
                        ps = pps.tile([P, KBX], F32, tag="proj")
                        ck_hold[(od, nb)] = ps
                    ps = ck_hold[(od, nb)]
                    nc.tensor.matmul(ps[:], ck_hold["wk"][:, c:c + 2, :],
                                     enc_sb[:, c:c + 2,
                                            nb * KBX:(nb + 1) * KBX],
                                     start=(c == 0), stop=(c == DC - 2),
                                     perf_mode=DR)
                return run

            def _ck_dq(od, nb):
                def run():
                    ps = ck_hold.pop((od, nb))
                    nc.vector.tensor_scalar(cksb[:, od, nb * KBX:(nb + 1) * KBX],
                                            ps[:], bc_sb["ca_dqk"][:, od:od + 1],
                                            bc_sb["ca_bk"][:, od:od + 1],
                                            Alu.mult, Alu.add)
                return run

            # 80 sub-unit closures consumed one per score-block/normalize slot
            ck_q = []
            for od in range(DC):
                for nb in range(S // KBX):
                    for c in range(0, DC, 2):
                        ck_q.append(_ck_step(od, nb, c))
                    ck_q.append(_ck_dq(od, nb))

            causal = cfg["self_mask"] and cfg.get("causal", False)
            sa_prm = dict(wq=w_d["sa_wq"], wk=w_d["sa_wk"], wv=w_d["sa_wv"],
                          wo=w_d["sa_wo"], bq=bc_sb["sa_bq"], bk=bc_sb["sa_bk"],
                          bo=bc_sb["sa_bo"], dqq=bc_sb["sa_dqq"],
                          dqk=bc_sb["sa_dqk"], dqv=bc_sb["sa_dqv"],
                          bv_row=bvrow_sb, fp8_pv=not cfg["self_mask"],
                          causal=causal, bg=ck_q)
            ca_prm = dict(wq=w_d["ca_wq"], wk=w_d["ca_wk"], wv=w_d["ca_wv"],
                          wo=w_d["ca_wo"], bq=bc_sb["ca_bq"], bk=bc_sb["ca_bk"],
                          bo=bc_sb["ca_bo"], dqq=bc_sb["ca_dqq"],
                          dqk=bc_sb["ca_dqk"], dqv=bc_sb["ca_dqv"],
                          bv_row=cvrow_sb, fp8_pv=not cfg["cross_mask"],
                          pre_kv=enc_sb, pre_ksb=cksb)

            # ---- self attention + LN1 ----
            h1 = pg.tile([P, DC, NQ], F32, tag="af32", bufs=2)
            cfg["_dbg_pfx"] = "sa"
            with tc.tile_pool(name="attn1", bufs=1) as pa:
                hbf, sq = _attention(nc, pa, pools, cfg, xT_d, T, xq_sb, mskT_d,
                                     xres_sb, sa_prm, h1, pg)
                y1 = pg.tile([P, DC, NQ], F32, tag="af32", bufs=2)
                y1b = pg.tile([P, DC, NQ], E4, tag="qsrc8", bufs=1)
                _layernorm(nc, pg, pools, cfg, h1, hbf, sq,
                           bc_sb["ln1_g"], bc_sb["ln1_b"], y1, y1b,
                           gq_c=bc_sb["ln1_gq"], bq_c=bc_sb["ln1_bq"])

            # ---- cross attention + LN2 ----
            h2 = pg.tile([P, DC, NQ], F32, tag="af32", bufs=2)
            cfg["_dbg_pfx"] = "ca"
            with tc.tile_pool(name="attn2", bufs=1) as pa:
                hbf, sq = _attention(nc, pa, pools, cfg, encT_d, S, y1b, emskT_d,
                                     y1, ca_prm, h2, pg)
                y2 = pg.tile([P, DC, NQ], F32, tag="af32", bufs=2)
                y2b = pg.tile([P, DC, NQ], BF, tag="qsrc", bufs=1)
                _layernorm(nc, pg, pools, cfg, h2, hbf, sq,
                           bc_sb["ln2_g"], bc_sb["ln2_b"], y2, y2b)

            if cfg.get("dbg"):
                for nm, t in (("h1", h1), ("y1", y1), ("h2", h2), ("y2", y2)):
                    d = dp(f"dbg_{nm}", list(t.shape), t.dtype, isOutput=True)
                    nc.sync.dma_start(d[:], t[:])

            # ---- FFN + LN3 ----
            with tc.tile_pool(name="ffn", bufs=1) as pa:
                fsb = pa.tile([P, FC, NQ], BF, tag="fsb")
                for ft in range(FC):
                    w1 = pw.tile([P, DC, P], BF, tag="wod", bufs=3)
                    nc.sync.dma_start(w1[:], f1_d[ft])
                    ps = pps.tile([P, NQ], F32, tag="proj")
                    for c in range(DC):
                        nc.tensor.matmul(ps[:], w1[:, c, :], y2b[:, c, :],
                                         start=(c == 0), stop=(c == DC - 1))
                    nc.scalar.activation(fsb[:, ft, :], ps[:], cfg["gelu"],
                                         bias=bc_sb["fc1_b"][:, ft:ft + 1])
                h3 = pg.tile([P, DC, NQ], F32, tag="af32", bufs=2)
                hbf, sq = [], []
                for od in range(DC):
                    w2 = pa.tile([P, FC, P], BF, tag="w2", bufs=2)
                    nc.sync.dma_start(w2[:], f2_d[od])
                    ps = pps.tile([P, NQ], F32, tag="proj")
                    for fc_ in range(FC):
                        nc.tensor.matmul(ps[:], w2[:, fc_, :], fsb[:, fc_, :],
                                         start=(fc_ == 0), stop=(fc_ == FC - 1))
                    nc.vector.scalar_tensor_tensor(h3[:, od, :], ps[:],
                                                   bc_sb["fc2_b"][:, od:od + 1],
                                                   y2[:, od, :], Alu.add, Alu.add)
                    hb = pg.tile([P, NQ], BF, tag="rot_hbf", bufs=2)
                    nc.scalar.copy(hb[:], h3[:, od, :])
                    s2 = pg.tile([P, NQ], BF, tag="rot_sq", bufs=2)
                    nc.vector.tensor_tensor(s2[:], hb[:], hb[:], Alu.mult)
                    hbf.append(hb)
                    sq.append(s2)
                out_f = pg.tile([P, DC, NQ], F32, tag="af32", bufs=2)
                _layernorm(nc, pg, pools, cfg, h3, hbf, sq,
                           bc_sb["ln3_g"], bc_sb["ln3_b"], out_f, None,
                           out_dram=outT_d.rearrange("(c p) n -> p c n", p=P))

    nc.compile()
    return nc


def make_in_maps(cfg, inputs):
    B, T, S, D, H, F = (cfg[k] for k in "BTSDHF")
    NQ = T // 2
    DC, FC = D // P, F // P
    HD = D // H
    bf = ml_dtypes.bfloat16

    def col(v):  # [D'] -> [P, D'//P]
        return np.ascontiguousarray(np.asarray(v, np.float32).reshape(-1, P).T)

    def wtile(w):  # [DO, DI] -> [DO/P, P, DI/P, P] od-tiles of transposed weight
        w = np.asarray(w, np.float32)
        do, di = w.shape
        return np.ascontiguousarray(
            w.reshape(do // P, P, di // P, P).transpose(0, 3, 2, 1)).astype(bf)

    def wtile8(w):  # fp8 od-tiles + per-output-row dequant (absmax/240)
        w = np.asarray(w, np.float32)
        do, di = w.shape
        am = np.abs(w).max(axis=1)
        s = 240.0 / np.maximum(am, 1e-30)
        w8 = (w * s[:, None]).astype(E4np)
        t = np.ascontiguousarray(
            w8.reshape(do // P, P, di // P, P).transpose(0, 3, 2, 1))
        return t, (1.0 / s).astype(np.float32)

    x_np = np.asarray(inputs["hidden_states"], np.float32)
    enc_np = np.asarray(inputs["encoder_hidden_states"], np.float32)
    s_x = 240.0 / max(float(np.abs(x_np).max()), 1e-30)
    s_enc = 240.0 / max(float(np.abs(enc_np).max()), 1e-30)

    shared = {}
    sc = HD ** -0.5
    t, dq = wtile8(np.asarray(inputs["sa_wq"]) * sc)
    cols = {}
    shared["sa_wqT"] = t
    cols["sa_dqq"] = col(dq / s_x)
    t, dq = wtile8(np.asarray(inputs["ca_wq"]) * sc)
    shared["ca_wqT"] = t
    cols["ca_dqq"] = col(dq / SY)
    t, dq = wtile8(inputs["sa_wk"])
    shared["sa_wkT"] = t
    cols["sa_dqk"] = col(dq / s_x)
    t, dq = wtile8(inputs["ca_wk"])
    shared["ca_wkT"] = t
    cols["ca_dqk"] = col(dq / s_enc)
    for nm in ("sa_wo", "ca_wo"):
        shared[nm + "T"] = wtile(inputs[nm])
    # V weights: per-tensor scale (dequant rides the free axis -> one scalar)
    wv = np.asarray(inputs["sa_wv"], np.float32)
    s_wv = 240.0 / max(float(np.abs(wv).max()), 1e-30)
    shared["sa_wvT"] = np.ascontiguousarray((wv.T * s_wv)).astype(E4np)
    cols["sa_dqv"] = np.full((P, 1), 1.0 / (s_wv * s_x), np.float32)
    wv = np.asarray(inputs["ca_wv"], np.float32)
    s_wv = 240.0 / max(float(np.abs(wv).max()), 1e-30)
    shared["ca_wvT"] = np.ascontiguousarray((wv.T * s_wv)).astype(E4np)
    cols["ca_dqv"] = np.full((P, 1), SV / (s_wv * s_enc), np.float32)
    shared["f1T"] = wtile(inputs["fc1_w"])
    shared["f2T"] = wtile(inputs["fc2_w"])
    cols["sa_bq"] = col(np.asarray(inputs["sa_bq"], np.float32) * sc)
    cols["ca_bq"] = col(np.asarray(inputs["ca_bq"], np.float32) * sc)
    for nm in ("sa_bk", "sa_bo", "ca_bk", "ca_bo", "fc2_b", "fc1_b",
               "ln1_g", "ln1_b", "ln2_g", "ln2_b", "ln3_g", "ln3_b"):
        cols[nm] = col(inputs[nm])
    cols["ln1_gq"] = col(np.asarray(inputs["ln1_g"], np.float32) * SY)
    cols["ln1_bq"] = col(np.asarray(inputs["ln1_b"], np.float32) * SY)
    shared["cols_blob"] = np.ascontiguousarray(
        np.concatenate([cols[nm] for nm, _ in COLS], axis=1))
    shared["sa_bv_row"] = np.asarray(inputs["sa_bv"], np.float32)[None, :].astype(bf)
    shared["ca_bv_row"] = (np.asarray(inputs["ca_bv"], np.float32)[None, :]
                           * SV).astype(bf)

    causal = cfg["self_mask"] and cfg.get("causal", False)
    in_maps = []
    for c in range(2 * B):
        b, half = divmod(c, 2)
        if causal:
            qs = np.concatenate([np.arange(blk * P, (blk + 1) * P)
                                 for blk in STRIPES[half]])
        else:
            qs = np.arange(half * NQ, (half + 1) * NQ)
        x = x_np[b]  # [T, D]
        m = {}
        m.update(shared)
        m["xT"] = np.ascontiguousarray((x.T * s_x)).astype(E4np)
        m["xqT"] = np.ascontiguousarray((x[qs].T * s_x)).astype(E4np)
        m["xres"] = np.ascontiguousarray(x[qs].T)
        m["encT"] = np.ascontiguousarray((enc_np[b].T * s_enc)).astype(E4np)
        if cfg.get("self_mask", True):
            m["emT"] = np.ascontiguousarray(np.exp(
                np.asarray(inputs["attention_mask"][b, 0], np.float32)[qs].T)).astype(bf)
        if cfg.get("cross_mask", False):
            m["cemT"] = np.ascontiguousarray(np.exp(
                np.asarray(inputs["encoder_attention_mask"][b, 0], np.float32)[qs].T)).astype(bf)
        in_maps.append(m)
    return in_maps


_NC_CACHE = {}


def get_nc(cfg=None):
    cfg = cfg or default_cfg()
    key = tuple(sorted((k, str(v)) for k, v in cfg.items()))
    if key not in _NC_CACHE:
        _NC_CACHE[key] = build_nc(dict(cfg))
    return _NC_CACHE[key]


def _is_causal_mask(mask, T):
    m = np.asarray(mask)
    tri = np.arange(T)[:, None] >= np.arange(T)[None, :]
    return bool(np.all((m[:, 0] > -1.0) == tri))


def kernel(**inputs):
    from concourse.bass_utils import run_bass_kernel_spmd

    cfg = default_cfg()
    cfg["self_mask"] = bool(np.any(np.asarray(inputs["attention_mask"])))
    cfg["cross_mask"] = bool(np.any(np.asarray(inputs["encoder_attention_mask"])))
    B, T, D = cfg["B"], cfg["T"], cfg["D"]
    NQ = T // 2
    cfg["causal"] = (cfg["self_mask"]
                     and _is_causal_mask(inputs["attention_mask"], T))
    causal = cfg["self_mask"] and cfg["causal"]
    nc = get_nc(cfg)
    in_maps = make_in_maps(cfg, inputs)
    res = run_bass_kernel_spmd(nc, in_maps, list(range(2 * B))).results
    out = np.empty((B, T, D), np.float32)
    for c in range(2 * B):
        b, half = divmod(c, 2)
        if causal:
            qi = np.concatenate([np.arange(blk * P, (blk + 1) * P)
                                 for blk in STRIPES[half]])
            out[b, qi, :] = res[c]["outT"].T
        else:
            out[b, half * NQ:(half + 1) * NQ, :] = res[c]["outT"].T
    return out
